# revision 1
# baseline (speedup 1.0000x reference)
"""Trainium2 Bass kernel for nn_Model4 (retrieval_knn).

Model: 3 l2-normalized feature streams -> 4 chained MultiheadAttention blocks
-> full = rt @ t_r.T -> per-group cosine logits [4, 256, 256].

Sharding (8 cores = 4 row-groups x 2 head-groups):
  core c = 2*g + j.  g in 0..3 owns rows R_g = [256g, 256g+256) (== final group g),
  j in 0..1 owns heads {2j, 2j+1} == feature columns [512j, 512j+512) of qkv space.

All activations are kept "feat-major" in SBUF: X.T as [feat(partition), rows(free)]
so every GEMM is a natural matmul without transposes (weights are host-transposed).
Attention uses transposed softmax (scoresT [S, L], no max subtraction -- scores are
~1e-3 magnitude) with column sums done via ones-vector matmuls on the PE.

Per MHA: K/V projections are computed S-sharded and AllGather'd across row-groups
(4-rank groups [[0,2,4,6],[1,3,5,7]]); attention context halves are exchanged
within the (g,*) pair (2-rank groups) before the (replicated) output projection.

Precision: weights + attention path in bf16 (fp32 PSUM accumulate); the l2-norm
statistics and final cosine/logits path stay in fp32(r).
"""
import sys

sys.path.insert(0, "/opt/trn_rl_repo")

import ml_dtypes
import numpy as np

import concourse.bass as bass  # noqa: F401
import concourse.tile as tile
import concourse.mybir as mybir
from concourse import bacc
from concourse.bass_utils import run_bass_kernel_spmd

E = 1024
P = 128
KO = E // P          # 8 feature chunks
RG = 256             # rows per group
NCORES = 8
PIECE = P * 4 * RG   # 131072 elements: [128,4,256] / [128,2,512] piece
F32 = mybir.dt.float32
F32R = mybir.dt.float32r
BF16 = mybir.dt.bfloat16
AF = mybir.ActivationFunctionType
GROUPS4 = [[0, 2, 4, 6], [1, 3, 5, 7]]   # gather S-shards across row-groups
GROUPS2 = [[0, 1], [2, 3], [4, 5], [6, 7]]  # exchange head halves within pair
EPS = 1e-8

_CACHE = {}


def build_nc():
    nc = bacc.Bacc("TRN2", target_bir_lowering=False, debug=False,
                   num_devices=NCORES)
    dram = {}

    def din(name, shape, dt=BF16):
        dram[name] = nc.dram_tensor(name, shape, dt, kind="ExternalInput").ap()

    # raw feature slices (feat-major, this core's 256 rows)
    din("x_text", [E, RG], F32)
    din("x_loc", [E, RG], F32)
    din("x_glob", [E, RG], F32)
    # full (replicated) projection weights, host-transposed to [in, out]
    for w in ("w_tl", "w_tg", "w_rep"):
        din(w, [E, E], F32R)
    for b in ("b_tl", "b_tg", "b_rep"):
        din(b, [E], F32)
    din("pos_l", [E], F32)
    din("pos_g", [E], F32)
    # per-MHA weights; q/k/v are this core's head-half [in, 512]
    for m in ("tl", "tg", "ff", "rt"):
        din(f"wq_{m}", [E, 512])
        din(f"wk_{m}", [E, 512])
        din(f"wv_{m}", [E, 512])
        din(f"wo_{m}", [E, E], F32R)
        din(f"bq_{m}", [512], F32)
        din(f"bk_{m}", [512], F32)
        din(f"bv_{m}", [512], F32)
        din(f"bo_{m}", [E], F32)

    out_logits = nc.dram_tensor("logits", [RG, RG], F32,
                                kind="ExternalOutput").ap()

    from contextlib import ExitStack
    with tile.TileContext(nc) as tc, ExitStack() as ctx:
        consts = ctx.enter_context(tc.tile_pool(name="consts", bufs=1))
        acts = ctx.enter_context(tc.tile_pool(name="acts", bufs=4))
        pers = ctx.enter_context(tc.tile_pool(name="pers", bufs=1))
        qps = ctx.enter_context(tc.tile_pool(name="qps", bufs=3))
        exps = ctx.enter_context(tc.tile_pool(name="exps", bufs=2))
        kpfp = ctx.enter_context(tc.tile_pool(name="kpfp", bufs=2))
        vpfp = ctx.enter_context(tc.tile_pool(name="vpfp", bufs=1))
        accs = ctx.enter_context(tc.tile_pool(name="accs", bufs=2))
        accfp = ctx.enter_context(tc.tile_pool(name="accfp", bufs=1))
        kvs = ctx.enter_context(tc.tile_pool(name="kvs", bufs=2))
        sqs = ctx.enter_context(tc.tile_pool(name="sqs", bufs=2))
        bcs = ctx.enter_context(tc.tile_pool(name="bcs", bufs=2))
        smalls = ctx.enter_context(tc.tile_pool(name="smalls", bufs=2))
        weights = ctx.enter_context(tc.tile_pool(name="weights", bufs=3))
        outs = ctx.enter_context(tc.tile_pool(name="outs", bufs=1))
        ps256 = ctx.enter_context(tc.tile_pool(name="ps256", bufs=3, space="PSUM"))
        ps512 = ctx.enter_context(tc.tile_pool(name="ps512", bufs=2, space="PSUM"))
        pssum = ctx.enter_context(tc.tile_pool(name="pssum", bufs=2, space="PSUM"))
        dram_p = ctx.enter_context(tc.tile_pool(name="dram_p", bufs=1, space="DRAM"))

        # ---------- constants ----------
        ones_cb = consts.tile([P, 1], BF16)
        nc.vector.memset(ones_cb, 1.0)
        # f32r ones for the fp32r norm path (memset can't write f32r)
        ones_cf = consts.tile([P, 1], F32)
        nc.vector.memset(ones_cf, 1.0)
        ones_col = consts.tile([P, 1], F32R)
        nc.vector.tensor_copy(ones_col, ones_cf)
        ones_rf = consts.tile([1, P], F32)
        nc.vector.memset(ones_rf, 1.0)
        ones_row = consts.tile([1, P], F32R)
        nc.vector.tensor_copy(ones_row, ones_rf)

        def load_bias_pp(name, n):
            """[n] dram -> [128, n//128] per-partition scalar layout."""
            t = consts.tile([P, n // P], F32, name=f"c_{name}")
            nc.sync.dma_start(t, dram[name].rearrange("(c p) -> p c", p=P))
            return t

        bias_pp = {}
        for nm in ("b_tl", "b_tg", "b_rep", "pos_l", "pos_g"):
            bias_pp[nm] = load_bias_pp(nm, E)
        for m in ("tl", "tg", "ff", "rt"):
            for bn in ("bq", "bk", "bv"):
                bias_pp[f"{bn}_{m}"] = load_bias_pp(f"{bn}_{m}", 512)
            bias_pp[f"bo_{m}"] = load_bias_pp(f"bo_{m}", E)

        # ---------- helpers ----------
        def load_w(name, half):
            """weight [1024, 512] (or half of [1024,1024]) -> [128,8,512]."""
            wdt = dram[name].dtype
            t = weights.tile([P, KO, 512], wdt, tag="w",
                             name=f"w_{name}_{half}",
                             padded_shape=[P, KO, 1024] if wdt == BF16 else None)
            src = dram[name]
            if src.shape[1] == E:
                src = src[:, half * 512:(half + 1) * 512]
            nc.sync.dma_start(t, src.rearrange("(ko p) c -> p ko c", p=P))
            return t

        def bcast_row(row_f32r, n):
            """[1, n] f32r -> [128, n] f32 broadcast via K=1 outer product."""
            ps = ps256.tile([P, n], F32, tag="mm", name="ps_bc")
            nc.tensor.matmul(ps, ones_row, row_f32r, start=True, stop=True)
            out = bcs.tile([P, n], F32, tag=f"bc{n}", name="bc")
            nc.any.tensor_copy(out=out, in_=ps)
            return out

        def gemm_fm(w_tiles, act, out, nco, bias=None, residual=None):
            """Feat-major GEMM: out[:, c, :] = sum_ko w[:, ko, c-chunk].T @ act[:, ko]
            w_tiles: list of [128, 8, 512] bf16 tiles covering nco*128 chans.
            act [128, 8, R] bf16; out [128, nco, R]; bias [128, nco] f32."""
            R = act.shape[2]
            for c in range(nco):
                w_sb = w_tiles[c // 4]
                cc = c % 4
                ps = ps256.tile([P, R], F32, tag="mm", name="ps_g")
                for ko in range(KO):
                    nc.tensor.matmul(ps, w_sb[:, ko, cc * P:(cc + 1) * P],
                                     act[:, ko], start=(ko == 0),
                                     stop=(ko == KO - 1))
                if bias is not None:
                    nc.vector.tensor_scalar_add(out[:, c], ps, bias[:, c:c + 1])
                    if residual is not None:
                        nc.vector.tensor_add(out[:, c], out[:, c],
                                             residual[:, c])
                elif residual is not None:
                    nc.vector.tensor_add(out[:, c], ps, residual[:, c])
                else:
                    nc.any.tensor_copy(out=out[:, c], in_=ps)

        def colsum_inv(src, nko, with_sqrt_eps=False):
            """src [128, nko, R]: per-free-column 1/||col||; returns [1, R] f32r."""
            R = src.shape[2]
            ps = pssum.tile([1, R], F32, tag="cs", name="ps_cs")
            for ko in range(nko):
                sq = sqs.tile([P, R], F32R, tag="sq", name="sq")
                nc.vector.tensor_mul(sq, src[:, ko].bitcast(F32),
                                     src[:, ko].bitcast(F32))
                nc.tensor.matmul(ps, ones_col, sq, start=(ko == 0),
                                 stop=(ko == nko - 1))
            inv = smalls.tile([1, R], F32R, tag="inv", name="inv")
            norm = smalls.tile([1, R], F32, tag="nrm", name="nrm")
            nc.scalar.sqrt(norm, ps)
            if with_sqrt_eps:
                nc.vector.tensor_scalar_max(norm, norm, EPS)
            with nc.allow_low_precision(reason="fp32r rounding intended"):
                nc.vector.reciprocal(inv, norm)
            return inv

        def attention(qp, kpf, vpf, acc_out, bv_pp):
            """qp [128,4,256] bf16; kpf [128,4(gs),4(dc),256] bf16;
            vpf [128,4(gs),2(sc),512] bf16; acc_out [128,4,256] bf16."""
            for h in range(2):
                expt = exps.tile([P, KO, RG], F32R, tag="exp", name=f"expt{h}")
                pss = pssum.tile([1, RG], F32, tag="cs", name="ps_sm")
                for s in range(8):
                    ps = ps256.tile([P, RG], F32, tag="mm", name="ps_sc")
                    for dk in range(2):
                        nc.tensor.matmul(
                            ps,
                            kpf[:, s // 2, 2 * h + dk,
                                (s % 2) * P:(s % 2 + 1) * P],
                            qp[:, 2 * h + dk],
                            start=(dk == 0), stop=(dk == 1))
                    nc.scalar.activation(expt[:, s], ps, AF.Exp, scale=0.0625)
                for s in range(8):
                    nc.tensor.matmul(pss, ones_col, expt[:, s],
                                     start=(s == 0), stop=(s == 7))
                inv = smalls.tile([1, RG], F32R, tag="inv", name="inv_sm")
                with nc.allow_low_precision(reason="fp32r rounding intended"):
                    nc.vector.reciprocal(inv, pss)
                bc = bcast_row(inv, RG)
                for dk in range(2):
                    ps = ps256.tile([P, RG], F32, tag="mm", name="ps_av")
                    for s in range(8):
                        nc.tensor.matmul(
                            ps,
                            vpf[:, s // 2, s % 2,
                                256 * h + P * dk:256 * h + P * (dk + 1)],
                            expt[:, s],
                            start=(s == 0), stop=(s == 7))
                    nc.vector.tensor_mul(acc_out[:, 2 * h + dk], ps, bc)
                    nc.vector.tensor_scalar_add(
                        acc_out[:, 2 * h + dk], acc_out[:, 2 * h + dk],
                        bv_pp[:, 2 * h + dk:2 * h + dk + 1])

        def kv_project(m, kv_src):
            """returns (kp [128,4,256] bf16, vp [128,2,512] bf16)."""
            wk = load_w(f"wk_{m}", 0)
            kp = kvs.tile([P, 4, RG], BF16, tag="kp", name=f"kp_{m}")
            gemm_fm([wk], kv_src, kp, 4, bias=bias_pp[f"bk_{m}"])
            wv = load_w(f"wv_{m}", 0)
            vp = kvs.tile([P, 2, 512], F32R, tag="vp", name=f"vp_{m}")
            for mc in range(2):
                ps = ps512.tile([P, 512], F32, tag="mm512", name="ps_vp")
                for ko in range(KO):
                    nc.tensor.matmul(ps, kv_src[:, ko, mc * P:(mc + 1) * P],
                                     wv[:, ko], start=(ko == 0),
                                     stop=(ko == KO - 1))
                nc.any.tensor_copy(out=vp[:, mc], in_=ps)
            return kp, vp

        def pack_piece(inbuf, off, sb_tile):
            if sb_tile.dtype == BF16 and inbuf.dtype != BF16:
                sb_tile = sb_tile.bitcast(F32R)
            shp = sb_tile.shape
            n = P * shp[1] * shp[2]
            nc.sync.dma_start(
                inbuf[off:off + n].rearrange("(p a b) -> p a b", p=P,
                                             a=shp[1]), sb_tile)

        def allgather(inbuf, outbuf, groups):
            nc.gpsimd.collective_compute(
                "AllGather", mybir.AluOpType.bypass,
                replica_groups=groups,
                ins=[inbuf.opt()], outs=[outbuf.opt()])

        def load_kv_full(outbuf, kp_off, vp_off, m):
            # kp piece: bf16 stored as f32r pairs (PIECE//2 f32r elems);
            # vp piece: native f32r (PIECE elems)
            kpf = kpfp.tile([P, 4, 4, RG], BF16, tag="kpf", name=f"kpf_{m}")
            vpf = vpfp.tile([P, 4, 2, 512], F32R, tag="vpf", name=f"vpf_{m}")
            for gs in range(4):
                nc.sync.dma_start(
                    kpf[:, gs].bitcast(F32R),
                    outbuf[gs, kp_off:kp_off + PIECE // 2].rearrange(
                        "(p a b) -> p a b", p=P, a=4))
                nc.sync.dma_start(
                    vpf[:, gs],
                    outbuf[gs, vp_off:vp_off + PIECE].rearrange(
                        "(p a b) -> p a b", p=P, a=2))
            return kpf, vpf

        def out_proj(m, outbuf2, acc_off, residual, out_tile):
            accf = accfp.tile([P, KO, RG], F32R, tag="accf", name=f"accf_{m}")
            for pos in range(2):
                nc.sync.dma_start(
                    accf[:, pos * 4:(pos + 1) * 4],
                    outbuf2[pos, acc_off:acc_off + PIECE].rearrange(
                        "(p a b) -> p a b", p=P, a=4))
            wo = [load_w(f"wo_{m}", 0), load_w(f"wo_{m}", 1)]
            gemm_fm(wo, accf, out_tile, 8, bias=bias_pp[f"bo_{m}"],
                    residual=residual)

        # ---------- stage 0: load + normalize ----------
        def load_raw(name):
            t = acts.tile([P, KO, RG], F32, tag="act", name=f"raw_{name}")
            nc.sync.dma_start(t, dram[name].rearrange("(ko p) r -> p ko r",
                                                      p=P))
            return t

        textT = load_raw("x_text")
        locT = load_raw("x_loc")
        globT = load_raw("x_glob")

        def normalize(raw, out, pos_pp=None):
            inv = colsum_inv(raw, KO)
            bc = bcast_row(inv, RG)
            for ko in range(KO):
                nc.vector.tensor_mul(out[:, ko], raw[:, ko], bc)
                if pos_pp is not None:
                    nc.vector.tensor_scalar_add(out[:, ko], out[:, ko],
                                                pos_pp[:, ko:ko + 1])

        # textn: f32r master (t_x GEMMs) + bf16 copy (q/k/v projections)
        textn = acts.tile([P, KO, RG], F32R, tag="act", name="textn")
        normalize(textT, textn)
        textn_bf = acts.tile([P, KO, RG], BF16, tag="actb", name="textn_bf")
        for ko in range(KO):
            nc.vector.tensor_copy(textn_bf[:, ko], textn[:, ko])
        localn = pers.tile([P, KO, RG], F32R, name="localn")
        normalize(locT, localn)
        kvl = acts.tile([P, KO, RG], BF16, tag="actb", name="kvl")
        for ko in range(KO):
            nc.vector.tensor_scalar_add(kvl[:, ko], localn[:, ko].bitcast(F32),
                                        bias_pp["pos_l"][:, ko:ko + 1])
        kvg = acts.tile([P, KO, RG], BF16, tag="actb", name="kvg")
        normalize(globT, kvg, pos_pp=bias_pp["pos_g"])

        # ---------- stage A: text projections ----------
        qp_tl = qps.tile([P, 4, RG], BF16, tag="qp", name="qp_tl")
        gemm_fm([load_w("wq_tl", 0)], textn_bf, qp_tl, 4, bias=bias_pp["bq_tl"])
        qp_tg = qps.tile([P, 4, RG], BF16, tag="qp", name="qp_tg")
        gemm_fm([load_w("wq_tg", 0)], textn_bf, qp_tg, 4, bias=bias_pp["bq_tg"])
        t_l = acts.tile([P, KO, RG], F32, tag="act", name="t_l")
        gemm_fm([load_w("w_tl", 0), load_w("w_tl", 1)], textn, t_l, 8,
                bias=bias_pp["b_tl"])
        t_g = acts.tile([P, KO, RG], F32, tag="act", name="t_g")
        gemm_fm([load_w("w_tg", 0), load_w("w_tg", 1)], textn, t_g, 8,
                bias=bias_pp["b_tg"])
        # t_r: f32r master (AG piece + fullT lhsT); bf16 copy for qp_rt
        t_r = acts.tile([P, KO, RG], F32R, tag="act", name="t_r")
        gemm_fm([load_w("w_rep", 0), load_w("w_rep", 1)], textn, t_r, 8,
                bias=bias_pp["b_rep"])
        t_r_bf = acts.tile([P, KO, RG], BF16, tag="actb", name="t_r_bf")
        for ko in range(KO):
            nc.vector.tensor_copy(t_r_bf[:, ko], t_r[:, ko])

        # ---------- stage B: tl + tg MHAs ----------
        kp_tl, vp_tl = kv_project("tl", kvl)
        kp_tg, vp_tg = kv_project("tg", kvg)
        in1 = dram_p.tile([3 * PIECE], F32R, name="in1")
        out1 = dram_p.tile([4, 3 * PIECE], F32R, name="out1")
        pack_piece(in1, 0, kp_tl)                      # PIECE//2
        pack_piece(in1, PIECE // 2, vp_tl)             # PIECE
        pack_piece(in1, 3 * PIECE // 2, kp_tg)         # PIECE//2
        pack_piece(in1, 2 * PIECE, vp_tg)              # PIECE
        allgather(in1, out1, GROUPS4)

        kpf_tl, vpf_tl = load_kv_full(out1, 0, PIECE // 2, "tl")
        acc_tl = accs.tile([P, 4, RG], F32R, tag="acc", name="acc_tl")
        attention(qp_tl, kpf_tl, vpf_tl, acc_tl, bias_pp["bv_tl"])
        kpf_tg, vpf_tg = load_kv_full(out1, 3 * PIECE // 2, 2 * PIECE, "tg")
        acc_tg = accs.tile([P, 4, RG], F32R, tag="acc", name="acc_tg")
        attention(qp_tg, kpf_tg, vpf_tg, acc_tg, bias_pp["bv_tg"])

        in2 = dram_p.tile([2 * PIECE], F32R, name="in2")
        out2 = dram_p.tile([2, 2 * PIECE], F32R, name="out2")
        pack_piece(in2, 0, acc_tl)
        pack_piece(in2, PIECE, acc_tg)
        allgather(in2, out2, GROUPS2)

        # lt / ff have residual uses -> keep f32 master + bf16 GEMM copy
        lt = acts.tile([P, KO, RG], F32, tag="act", name="lt")
        out_proj("tl", out2, 0, t_l, lt)
        gt = acts.tile([P, KO, RG], BF16, tag="actb", name="gt")
        out_proj("tg", out2, PIECE, t_g, gt)
        lt_bf = acts.tile([P, KO, RG], BF16, tag="actb", name="lt_bf")
        for ko in range(KO):
            nc.vector.tensor_copy(lt_bf[:, ko], lt[:, ko])

        # ---------- stage C: ff MHA (q=lt, kv=gt) ----------
        qp_ff = qps.tile([P, 4, RG], BF16, tag="qp", name="qp_ff")
        gemm_fm([load_w("wq_ff", 0)], lt_bf, qp_ff, 4, bias=bias_pp["bq_ff"])
        kp_ff, vp_ff = kv_project("ff", gt)
        in3 = dram_p.tile([3 * PIECE // 2], F32R, name="in3")
        out3 = dram_p.tile([4, 3 * PIECE // 2], F32R, name="out3")
        pack_piece(in3, 0, kp_ff)
        pack_piece(in3, PIECE // 2, vp_ff)
        allgather(in3, out3, GROUPS4)

        kpf_ff, vpf_ff = load_kv_full(out3, 0, PIECE // 2, "ff")
        acc_ff = accs.tile([P, 4, RG], F32R, tag="acc", name="acc_ff")
        attention(qp_ff, kpf_ff, vpf_ff, acc_ff, bias_pp["bv_ff"])
        in4 = dram_p.tile([PIECE], F32R, name="in4")
        out4 = dram_p.tile([2, PIECE], F32R, name="out4")
        pack_piece(in4, 0, acc_ff)
        allgather(in4, out4, GROUPS2)
        ff = acts.tile([P, KO, RG], BF16, tag="actb", name="ff")
        out_proj("ff", out4, 0, lt, ff)

        # ---------- stage D: rt MHA (q=t_r, kv=ff) ----------
        qp_rt = qps.tile([P, 4, RG], BF16, tag="qp", name="qp_rt")
        gemm_fm([load_w("wq_rt", 0)], t_r_bf, qp_rt, 4, bias=bias_pp["bq_rt"])
        kp_rt, vp_rt = kv_project("rt", ff)
        in5 = dram_p.tile([7 * PIECE // 2], F32R, name="in5")
        out5 = dram_p.tile([4, 7 * PIECE // 2], F32R, name="out5")
        pack_piece(in5, 0, kp_rt)                     # PIECE//2
        pack_piece(in5, PIECE // 2, vp_rt)            # PIECE
        pack_piece(in5, 3 * PIECE // 2, t_r)          # 2*PIECE
        allgather(in5, out5, GROUPS4)

        kpf_rt, vpf_rt = load_kv_full(out5, 0, PIECE // 2, "rt")
        acc_rt = accs.tile([P, 4, RG], F32R, tag="acc", name="acc_rt")
        attention(qp_rt, kpf_rt, vpf_rt, acc_rt, bias_pp["bv_rt"])
        in6 = dram_p.tile([PIECE], F32R, name="in6")
        out6 = dram_p.tile([2, PIECE], F32R, name="out6")
        pack_piece(in6, 0, acc_rt)
        allgather(in6, out6, GROUPS2)
        rt = acts.tile([P, KO, RG], F32R, tag="act", name="rt")
        out_proj("rt", out6, 0, None, rt)

        # ---------- stage E: full = rt @ t_r.T, cosine logits ----------
        fullT = acts.tile([P, KO, RG], F32, tag="act", name="fullT")
        for gs in range(4):
            trf = exps.tile([P, KO, RG], F32R, tag="exp", name=f"trf{gs}")
            nc.sync.dma_start(
                trf, out5[gs, 3 * PIECE // 2:7 * PIECE // 2].rearrange(
                    "(p a b) -> p a b", p=P, a=KO))
            for mh in range(2):
                mc = gs * 2 + mh
                ps = ps256.tile([P, RG], F32, tag="mm", name="ps_full")
                for ko in range(KO):
                    nc.tensor.matmul(ps, trf[:, ko, mh * P:(mh + 1) * P],
                                     rt[:, ko], start=(ko == 0),
                                     stop=(ko == KO - 1))
                nc.any.tensor_copy(out=fullT[:, mc], in_=ps)

        inv_full = colsum_inv(fullT, KO, with_sqrt_eps=True)
        bc_full = bcast_row(inv_full, RG)
        ffn = acts.tile([P, KO, RG], F32R, tag="act", name="ffn")
        for ko in range(KO):
            nc.vector.tensor_mul(ffn[:, ko], fullT[:, ko], bc_full)

        lg = outs.tile([P, 2, RG], F32, name="lg")
        for lc in range(2):
            ps = ps256.tile([P, RG], F32, tag="mm", name="ps_lg")
            for ko in range(KO):
                nc.tensor.matmul(ps, ffn[:, ko, lc * P:(lc + 1) * P],
                                 localn[:, ko], start=(ko == 0),
                                 stop=(ko == KO - 1))
            nc.any.tensor_copy(out=lg[:, lc], in_=ps)
        nc.sync.dma_start(out_logits.rearrange("(lc p) q -> p lc q", p=P), lg)

    nc.compile()
    return nc


def make_in_maps(local_feat, global_feat, text_feat,
                 w_tl, b_tl, w_tg, b_tg, w_rep, b_rep,
                 pos_local, pos_global, mha_params):
    """mha_params: dict m -> (wi, bi, wo, bo)."""
    f32 = np.float32
    bf16 = ml_dtypes.bfloat16
    textT = np.ascontiguousarray(text_feat.T.astype(f32))
    locT = np.ascontiguousarray(local_feat.T.astype(f32))
    globT = np.ascontiguousarray(global_feat.T.astype(f32))
    shared = {
        "w_tl": np.ascontiguousarray(w_tl.T.astype(f32)),
        "w_tg": np.ascontiguousarray(w_tg.T.astype(f32)),
        "w_rep": np.ascontiguousarray(w_rep.T.astype(f32)),
        "b_tl": b_tl.astype(f32), "b_tg": b_tg.astype(f32),
        "b_rep": b_rep.astype(f32),
        "pos_l": pos_local.astype(f32), "pos_g": pos_global.astype(f32),
    }
    per_j = {}
    for j in range(2):
        d = {}
        for m, (wi, bi, wo, bo) in mha_params.items():
            sl = slice(512 * j, 512 * (j + 1))
            d[f"wq_{m}"] = np.ascontiguousarray(wi[0 * E:1 * E][sl].T.astype(bf16))
            d[f"wk_{m}"] = np.ascontiguousarray(wi[1 * E:2 * E][sl].T.astype(bf16))
            d[f"wv_{m}"] = np.ascontiguousarray(wi[2 * E:3 * E][sl].T.astype(bf16))
            d[f"wo_{m}"] = np.ascontiguousarray(wo.T.astype(f32))
            d[f"bq_{m}"] = bi[0 * E:1 * E][sl].astype(f32)
            d[f"bk_{m}"] = bi[1 * E:2 * E][sl].astype(f32)
            d[f"bv_{m}"] = bi[2 * E:3 * E][sl].astype(f32)
            d[f"bo_{m}"] = bo.astype(f32)
        per_j[j] = d

    in_maps = []
    for c in range(NCORES):
        g, j = c // 2, c % 2
        rs = slice(RG * g, RG * (g + 1))
        m = {
            "x_text": np.ascontiguousarray(textT[:, rs]),
            "x_loc": np.ascontiguousarray(locT[:, rs]),
            "x_glob": np.ascontiguousarray(globT[:, rs]),
        }
        m.update(shared)
        m.update(per_j[j])
        in_maps.append(m)
    return in_maps


def kernel(local_feat, global_feat, text_feat,
           w_tl, b_tl, w_tg, b_tg, w_rep, b_rep,
           pos_local, pos_global,
           tl_wi, tl_bi, tl_wo, tl_bo,
           tg_wi, tg_bi, tg_wo, tg_bo,
           ff_wi, ff_bi, ff_wo, ff_bo,
           rt_wi, rt_bi, rt_wo, rt_bo,
           n_groups):
    assert int(n_groups) == 4
    if "nc" not in _CACHE:
        _CACHE["nc"] = build_nc()
    nc = _CACHE["nc"]
    mha_params = {
        "tl": (tl_wi, tl_bi, tl_wo, tl_bo),
        "tg": (tg_wi, tg_bi, tg_wo, tg_bo),
        "ff": (ff_wi, ff_bi, ff_wo, ff_bo),
        "rt": (rt_wi, rt_bi, rt_wo, rt_bo),
    }
    in_maps = make_in_maps(np.asarray(local_feat), np.asarray(global_feat),
                           np.asarray(text_feat),
                           np.asarray(w_tl), np.asarray(b_tl),
                           np.asarray(w_tg), np.asarray(b_tg),
                           np.asarray(w_rep), np.asarray(b_rep),
                           np.asarray(pos_local), np.asarray(pos_global),
                           {k: tuple(np.asarray(x) for x in v)
                            for k, v in mha_params.items()})
    res = run_bass_kernel_spmd(nc, in_maps, core_ids=list(range(NCORES)))
    _CACHE["last_results"] = res
    out = np.empty((4, RG, RG), dtype=np.float32)
    for g in range(4):
        out[g] = res.results[2 * g]["logits"]
    return out



# revision 14
# speedup vs baseline: 1.8678x; 1.8678x over previous
"""Trainium2 Bass kernel for nn_Model4 (retrieval_knn).

Model: 3 l2-normalized feature streams -> 4 chained MultiheadAttention blocks
-> full = rt @ t_r.T -> per-group cosine logits [4, 256, 256].

Sharding (8 cores = 4 row-groups x 2 head-halves): core c = 2*g + j.
g owns rows R_g = [256g, 256g+256) (== final group g); j owns qkv feature
columns [512j, 512j+512) for the ff/rt MHAs only.

The collective cost (15us constant + bytes/40GBps, serialized on one device)
dominates, so the design minimizes collectives:
  - tl/tg MHAs: fully replicated within the pair (all 4 heads; K/V computed
    locally over the full sequence from the replicated local/global streams).
    No collective at all for these two MHAs.
  - t_r: computed in full on every core (a cheap GEMM beats a gather).
  - ff/rt MHAs: head-half sharded; K/V projections S-sharded then quad
    AllGather (bf16); context halves pair AllGather (bf16).
=> 4 collectives total vs the previous 6.

All GEMMs bf16 x bf16 with fp32 PSUM accumulation; the K-projection bias is
dropped (softmax is invariant to per-query constant score shifts); V bias is
added to the context after attention (attention weights sum to 1).  The
Activation engine uses only {square, ln, exp, copy} (one act table set ->
one table load); 1/sqrt(x) is computed as exp(-0.5*ln(x)).

Engine queues: SP = input/weight loads + output store; DVE = scales, bias
adds, context muls, collective pack DMAs and (deferred) unpack DMAs;
Act = squares/exp/ln + PSUM->SBUF copies of K/V; Pool = collectives and a
few adds.  Collective unpack DMAs are issued late in program order so they
never head-of-line block work that should run under the AllGather.
"""
import os
import sys

sys.path.insert(0, "/opt/trn_rl_repo")

import ml_dtypes
import numpy as np

import concourse.bass as bass  # noqa: F401
import concourse.tile as tile
import concourse.mybir as mybir
from concourse import bacc
from concourse.bass_utils import run_bass_kernel_spmd

E = 1024
P = 128
KO = E // P          # 8 feature chunks
RG = 256             # rows per group
NCORES = 8
F32 = mybir.dt.float32
F32R = mybir.dt.float32r
BF16 = mybir.dt.bfloat16
AF = mybir.ActivationFunctionType
GROUPS4 = [[0, 2, 4, 6], [1, 3, 5, 7]]   # gather S-shards across row-groups
GROUPS2 = [[0, 1], [2, 3], [4, 5], [6, 7]]  # exchange head halves within pair
EPS = 1e-8
KV_ELEMS = 512 * RG          # bf16 elems of one kp or vp piece
CTX_ELEMS = 512 * RG         # bf16 elems of one ctx piece

_CACHE = {}


def build_nc():
    nc = bacc.Bacc("TRN2", target_bir_lowering=False, debug=False,
                   num_devices=NCORES)
    dram = {}

    def din(name, shape, dt=BF16):
        dram[name] = nc.dram_tensor(name, shape, dt, kind="ExternalInput").ap()

    # full feature streams (feat-major: [feat, row]) + own-row slices
    din("x_text", [E, E])
    din("x_loc", [E, E])
    din("x_glob", [E, E])
    din("x_text_own", [E, RG])
    din("x_loc_own", [E, RG])
    for w in ("w_tl", "w_tg", "w_rep"):
        din(w, [E, E])
    for b in ("b_tl", "b_tg", "b_rep"):
        din(b, [E], F32)
    din("pos_l", [E], F32)
    din("pos_g", [E], F32)
    # tl/tg: full-head q/k/v weights; ff/rt: this core's head-half
    for m in ("tl", "tg"):
        din(f"wq_{m}", [E, E])
        din(f"wk_{m}", [E, E])
        din(f"wv_{m}", [E, E])
        din(f"wo_{m}", [E, E])
        din(f"bq_{m}", [E], F32)
        din(f"bv_{m}", [E], F32)
        din(f"bo_{m}", [E], F32)
    for m in ("ff", "rt"):
        din(f"wq_{m}", [E, 512])
        din(f"wk_{m}", [E, 512])
        din(f"wv_{m}", [E, 512])
        din(f"wo_{m}", [E, E])
        din(f"bq_{m}", [512], F32)
        din(f"bv_{m}", [512], F32)
        din(f"bo_{m}", [E], F32)

    out_logits = nc.dram_tensor("logits", [RG, RG], F32,
                                kind="ExternalOutput").ap()
    dbg_names = [x for x in os.environ.get("KDEBUG", "").split(",") if x]
    dbg_outs = {}

    from contextlib import ExitStack
    with tile.TileContext(nc) as tc, ExitStack() as ctx:
        consts = ctx.enter_context(tc.tile_pool(name="consts", bufs=1))
        streams = ctx.enter_context(tc.tile_pool(name="streams", bufs=2))
        kvfull = ctx.enter_context(tc.tile_pool(name="kvfull", bufs=2))
        wfull = ctx.enter_context(tc.tile_pool(name="wfull", bufs=2))
        whalf = ctx.enter_context(tc.tile_pool(name="whalf", bufs=2))
        acts = ctx.enter_context(tc.tile_pool(name="acts", bufs=4))
        pers = ctx.enter_context(tc.tile_pool(name="pers", bufs=1))
        qps = ctx.enter_context(tc.tile_pool(name="qps", bufs=2))
        exps = ctx.enter_context(tc.tile_pool(name="exps", bufs=1))
        sqs = ctx.enter_context(tc.tile_pool(name="sqs", bufs=2))
        ctxs = ctx.enter_context(tc.tile_pool(name="ctxs", bufs=1))
        bcs = ctx.enter_context(tc.tile_pool(name="bcs", bufs=1))
        smalls = ctx.enter_context(tc.tile_pool(name="smalls", bufs=1))
        outs = ctx.enter_context(tc.tile_pool(name="outs", bufs=1))
        ps512 = ctx.enter_context(tc.tile_pool(name="ps512", bufs=2,
                                               space="PSUM"))
        ps256 = ctx.enter_context(tc.tile_pool(name="ps256", bufs=3,
                                               space="PSUM"))
        pssum = ctx.enter_context(tc.tile_pool(name="pssum", bufs=1,
                                               space="PSUM"))
        dram_p = ctx.enter_context(tc.tile_pool(name="dram_p", bufs=1,
                                                space="DRAM"))

        # ---------- constants ----------
        ones_cb = consts.tile([P, 1], BF16)
        nc.vector.memset(ones_cb, 1.0)
        ones_cf = consts.tile([P, 1], F32)
        nc.vector.memset(ones_cf, 1.0)
        ones_col = consts.tile([P, 1], F32R)
        nc.vector.tensor_copy(ones_col, ones_cf)
        ones_row_f = consts.tile([1, P], F32)
        nc.vector.memset(ones_row_f, 1.0)
        ones_row = consts.tile([1, P], F32R)
        nc.vector.tensor_copy(ones_row, ones_row_f)

        def load_bias_pp(name, n):
            """[n] dram -> [128, n//128] per-partition scalar layout."""
            t = consts.tile([P, n // P], F32, name=f"c_{name}")
            nc.sync.dma_start(t, dram[name].rearrange("(c p) -> p c", p=P))
            return t

        bias_pp = {}
        for nm in ("b_tl", "b_tg", "b_rep", "pos_l", "pos_g"):
            bias_pp[nm] = load_bias_pp(nm, E)
        for m in ("tl", "tg"):
            for bn in ("bq", "bv", "bo"):
                bias_pp[f"{bn}_{m}"] = load_bias_pp(f"{bn}_{m}", E)
        for m in ("ff", "rt"):
            for bn in ("bq", "bv"):
                bias_pp[f"{bn}_{m}"] = load_bias_pp(f"{bn}_{m}", 512)
            bias_pp[f"bo_{m}"] = load_bias_pp(f"bo_{m}", E)

        # ---------- debug ----------
        def _dbg(nm, t):
            if nm not in dbg_names:
                return
            do = nc.dram_tensor(f"dbg_{nm}", [P] + list(t.shape[1:]),
                                t.dtype, kind="ExternalOutput").ap()
            nc.sync.dma_start(do, t)

        # ---------- helpers ----------
        def load_w(name, pool, nco):
            """weight [1024, nco*128] dram -> [128, 8, nco*128] bf16 tile."""
            t = pool.tile([P, KO, nco * P], BF16, tag="w", name=f"w_{name}")
            nc.sync.dma_start(t, dram[name].rearrange("(ko p) c -> p ko c",
                                                      p=P))
            return t

        def colsum_inv(src, nko, with_eps=False):
            """src [128, nko, R] bf16: per-free-column 1/||col||, [1,R] f32r."""
            R = src.shape[2]
            inv = smalls.tile([1, R], F32R, tag=f"inv{R}", name="inv")
            for h in range(0, R, 512):
                w = min(512, R - h)
                ps = pssum.tile([1, w], F32, tag=f"cs{w}", name="ps_cs")
                for ko in range(nko):
                    sq = sqs.tile([P, w], F32R, tag=f"sq{w}", name="sq")
                    nc.scalar.activation(sq, src[:, ko, h:h + w], AF.Square)
                    nc.tensor.matmul(ps, ones_col, sq, start=(ko == 0),
                                     stop=(ko == nko - 1))
                src_ps = ps
                if with_eps:
                    mx = smalls.tile([1, w], F32, tag=f"mx{w}", name="mx")
                    nc.vector.tensor_scalar_max(mx, ps, EPS * EPS)
                    src_ps = mx
                rec = smalls.tile([1, w], F32, tag=f"rc{w}", name="rec")
                with nc.allow_low_precision(reason="fp32 reciprocal+sqrt"):
                    nc.vector.reciprocal(rec, src_ps)
                nc.scalar.activation(inv[:, h:h + w], rec, AF.Sqrt)
            return inv

        def bcast_row(row_f32r, n):
            """[1, n] f32r -> [128, n] f32 broadcast via K=1 outer product."""
            out = bcs.tile([P, n], F32, tag=f"bc{n}", name="bc")
            for h in range(0, n, 512):
                w = min(512, n - h)
                pool, tag = (ps256, "mm") if w <= 256 else (ps512, "mm512")
                ps = pool.tile([P, w], F32, tag=tag, name="ps_bc")
                nc.tensor.matmul(ps, ones_row, row_f32r[:, h:h + w],
                                 start=True, stop=True)
                nc.vector.tensor_copy(out[:, h:h + w], ps)
            return out

        def gemm_fm(w_sb, act, out, nco, bias=None, residual=None,
                    eng=None, act_copy=False):
            """Feat-major GEMM: out[:, c, :] = w[:, :, c*128:].T @ act (+b+r).
            w_sb [128, 8, nco*128] bf16; act [128, 8, R] bf16;
            out [128, nco, R]; bias [128, >=nco] f32; residual bf16."""
            R = act.shape[2]
            eng = eng or nc.vector
            for c in range(nco):
                for h in range(0, R, 512):
                    w = min(512, R - h)
                    pool, tag = (ps256, "mm") if w <= 256 else (ps512, "mm512")
                    ps = pool.tile([P, w], F32, tag=tag, name="ps_g")
                    for ko in range(KO):
                        nc.tensor.matmul(ps, w_sb[:, ko, c * P:(c + 1) * P],
                                         act[:, ko, h:h + w], start=(ko == 0),
                                         stop=(ko == KO - 1))
                    o = out[:, c, h:h + w]
                    if bias is not None:
                        eng.tensor_scalar_add(o, ps, bias[:, c:c + 1])
                        if residual is not None:
                            nc.gpsimd.tensor_add(o, o, residual[:, c, h:h + w])
                    elif residual is not None:
                        eng.tensor_scalar_add(o, ps, 0.0)
                        nc.gpsimd.tensor_add(o, o, residual[:, c, h:h + w])
                    elif act_copy:
                        nc.scalar.copy(o, ps)
                    else:
                        eng.tensor_copy(o, ps)

        def vproj_smajor(w_sb, act, vp, nchan):
            """S-major V projection: vp[:, s, :] = act[:, :, s128].T @ w.
            act [128, 8, S] bf16 feat-major; w_sb [128, 8, nchan];
            vp [128, S//128, nchan] bf16 (no bias: bv folds into ctx)."""
            S = act.shape[2]
            for s in range(S // P):
                for h in range(0, nchan, 512):
                    w = min(512, nchan - h)
                    ps = ps512.tile([P, w], F32, tag="mm512", name="ps_v")
                    for ko in range(KO):
                        nc.tensor.matmul(ps, act[:, ko, s * P:(s + 1) * P],
                                         w_sb[:, ko, h:h + w],
                                         start=(ko == 0), stop=(ko == KO - 1))
                    nc.scalar.copy(vp[:, s, h:h + w], ps)

        def attention(qp, kp_sl, vp_sl, ctx_out, bv_pp, nheads):
            """qp [128,2*nheads,256] bf16 feat-major.
            kp_sl(hc, s) -> [128,128] bf16 lhsT (proj-chan chunk hc, S chunk s)
            vp_sl(s, cc) -> [128,128] bf16 lhsT (S chunk s, ctx chunk cc).
            ctx_out [128,2*nheads,256] bf16.  d=256 per head (2 chunks)."""
            ns = 8
            for h in range(nheads):
                expt = exps.tile([P, ns, RG], BF16, tag="exp", name="expt")
                pss = pssum.tile([1, RG], F32, tag="cs256", name="ps_sm")
                for s in range(ns):
                    ps = ps256.tile([P, RG], F32, tag="mm", name="ps_sc")
                    for dk in range(2):
                        nc.tensor.matmul(ps, kp_sl(2 * h + dk, s),
                                         qp[:, 2 * h + dk],
                                         start=(dk == 0), stop=(dk == 1))
                    nc.scalar.activation(expt[:, s], ps, AF.Exp, scale=0.0625)
                for s in range(ns):
                    nc.tensor.matmul(pss, ones_cb, expt[:, s],
                                     start=(s == 0), stop=(s == ns - 1))
                inv = smalls.tile([1, RG], F32R, tag="invsm", name="inv_sm")
                with nc.allow_low_precision(reason="fp32r rounding intended"):
                    nc.vector.reciprocal(inv, pss)
                bc = bcast_row(inv, RG)
                for dk in range(2):
                    cc = 2 * h + dk
                    ps = ps256.tile([P, RG], F32, tag="mm", name="ps_av")
                    for s in range(ns):
                        nc.tensor.matmul(ps, vp_sl(s, cc), expt[:, s],
                                         start=(s == 0), stop=(s == ns - 1))
                    nc.vector.tensor_mul(ctx_out[:, cc], ps, bc)
                    nc.gpsimd.tensor_scalar_add(ctx_out[:, cc],
                                                ctx_out[:, cc],
                                                bv_pp[:, cc:cc + 1])

        def pack_piece(inbuf, off, sb_tile):
            """bf16 SBUF tile -> bf16 dram flat buffer."""
            shp = sb_tile.shape
            n = P * shp[1] * shp[2]
            nc.sync.dma_start(
                inbuf[off:off + n].rearrange("(p a b) -> p a b", p=P,
                                             a=shp[1]), sb_tile)

        def allgather(inbuf, outbuf, groups):
            nc.gpsimd.collective_compute(
                "AllGather", mybir.AluOpType.bypass,
                replica_groups=groups,
                ins=[inbuf.opt()], outs=[outbuf.opt()])

        def kv_gather_issue(m, kv_src):
            """S-shard k/v projection for this core's head-half + quad AG."""
            wk = load_w(f"wk_{m}", whalf, 4)
            kp = ctxs.tile([P, 4, RG], BF16, tag="kp", name=f"kp_{m}")
            gemm_fm(wk, kv_src, kp, 4, act_copy=True)
            wv = load_w(f"wv_{m}", whalf, 4)
            vp = ctxs.tile([P, 2, 512], BF16, tag="vp", name=f"vp_{m}")
            vproj_smajor(wv, kv_src, vp, 512)
            inb = dram_p.tile([2 * KV_ELEMS], BF16, name=f"in_{m}")
            outb = dram_p.tile([4, 2 * KV_ELEMS], BF16, name=f"out_{m}")
            _dbg(f"kp_{m}_piece", kp)
            _dbg(f"vp_{m}_piece", vp)
            pack_piece(inb, 0, kp)
            pack_piece(inb, KV_ELEMS, vp)
            if f"packback_{m}" in dbg_names:
                t_pb = consts.tile([P, 4, RG], BF16, name=f"t_pb_{m}")
                nc.sync.dma_start(
                    t_pb,
                    inb[0:KV_ELEMS].rearrange("(p a b) -> p a b", p=P, a=4))
                _dbg(f"packback_{m}", t_pb)
            allgather(inb, outb, GROUPS4)
            return outb

        def kv_gather_unpack(m, outb):
            """Deferred unpack of the quad-AG result into full-S k/v tiles."""
            kpf = kvfull.tile([P, 4, 4, RG], BF16, tag="kv", name=f"kpf_{m}",
                              padded_shape=[P, 4, 4, 2 * RG])
            vpf = kvfull.tile([P, 4, 2, 512], BF16, tag="kv", name=f"vpf_{m}",
                              padded_shape=[P, 4, 2, 1024])
            for gs in range(4):
                nc.sync.dma_start(
                    kpf[:, gs],
                    outb[gs, 0:KV_ELEMS].rearrange("(p a b) -> p a b",
                                                   p=P, a=4))
                nc.sync.dma_start(
                    vpf[:, gs],
                    outb[gs, KV_ELEMS:].rearrange("(p a b) -> p a b",
                                                  p=P, a=2))
            _dbg(f"kpf_{m}", kpf)
            _dbg(f"vpf_{m}", vpf)
            kp_sl = lambda hc, s: kpf[:, s // 2, hc, (s % 2) * P:(s % 2 + 1) * P]
            vp_sl = lambda s, cc: vpf[:, s // 2, s % 2, cc * P:(cc + 1) * P]
            return kp_sl, vp_sl

        def ctx_gather_issue(m, ctx_half):
            inb = dram_p.tile([CTX_ELEMS], BF16, name=f"inc_{m}")
            outb = dram_p.tile([2, CTX_ELEMS], BF16, name=f"outc_{m}")
            pack_piece(inb, 0, ctx_half)
            allgather(inb, outb, GROUPS2)
            return outb

        def ctx_gather_unpack(m, outb):
            full = ctxs.tile([P, KO, RG], BF16, tag="cf", name=f"ctxf_{m}")
            for r in range(2):
                nc.sync.dma_start(
                    full[:, 4 * r:4 * r + 4],
                    outb[r].rearrange("(p a b) -> p a b", p=P, a=4))
            return full

        # ---------- stage 0: input streams ----------
        def load_stream(name):
            t = streams.tile([P, KO, E], BF16, tag="x", name=name)
            nc.sync.dma_start(t, dram[name].rearrange("(ko p) r -> p ko r",
                                                      p=P))
            return t

        kvg = load_stream("x_glob")            # becomes kvg in place
        kvl = load_stream("x_loc")             # becomes kvl in place
        t_own = pers.tile([P, KO, RG], BF16, name="textn_own")
        nc.sync.dma_start(t_own, dram["x_text_own"].rearrange(
            "(ko p) r -> p ko r", p=P))
        l_own = pers.tile([P, KO, RG], BF16, name="localn_own")
        nc.sync.dma_start(l_own, dram["x_loc_own"].rearrange(
            "(ko p) r -> p ko r", p=P))

        # normalize glob first (feeds the tg chain = critical path)
        inv_g = colsum_inv(kvg, KO)
        bc_g = bcast_row(inv_g, E)
        for ko in range(KO):
            nc.vector.tensor_mul(kvg[:, ko], kvg[:, ko], bc_g)
            nc.gpsimd.tensor_scalar_add(kvg[:, ko], kvg[:, ko],
                                        bias_pp["pos_g"][:, ko:ko + 1])
        # local stream + own-row slices (off the critical path engines-wise)
        inv_l = colsum_inv(kvl, KO)
        bc_l = bcast_row(inv_l, E)
        for ko in range(KO):
            nc.vector.tensor_mul(kvl[:, ko], kvl[:, ko], bc_l)
            nc.gpsimd.tensor_scalar_add(kvl[:, ko], kvl[:, ko],
                                        bias_pp["pos_l"][:, ko:ko + 1])
        inv_to = colsum_inv(t_own, KO)
        bc_to = bcast_row(inv_to, RG)
        for ko in range(KO):
            nc.vector.tensor_mul(t_own[:, ko], t_own[:, ko], bc_to)
        inv_lo = colsum_inv(l_own, KO)
        bc_lo = bcast_row(inv_lo, RG)
        for ko in range(KO):
            nc.vector.tensor_mul(l_own[:, ko], l_own[:, ko], bc_lo)
        # full-text norms, computed early via chunked DMA so every Rsqrt
        # happens before the first Exp (act tables: one switch each way)
        inv_t = smalls.tile([1, E], F32R, tag="inv1024", name="inv_t")
        for h in range(2):
            ps = pssum.tile([1, 512], F32, tag="cs512", name="ps_xt")
            for ko in range(KO):
                stg = sqs.tile([P, 512], BF16, tag="stg", name="stg")
                nc.sync.dma_start(
                    stg, dram["x_text"][ko * P:(ko + 1) * P,
                                        h * 512:(h + 1) * 512])
                sq = sqs.tile([P, 512], F32R, tag="sq512", name="sq")
                nc.scalar.activation(sq, stg, AF.Square)
                nc.tensor.matmul(ps, ones_col, sq, start=(ko == 0),
                                 stop=(ko == KO - 1))
            rec = smalls.tile([1, 512], F32, tag="rc512", name="rec_t")
            with nc.allow_low_precision(reason="fp32 reciprocal+sqrt"):
                nc.vector.reciprocal(rec, ps)
            nc.scalar.activation(inv_t[:, h * 512:(h + 1) * 512], rec,
                                 AF.Sqrt)

        _dbg("kvg", kvg)
        _dbg("kvl", kvl)
        _dbg("t_own", t_own)
        _dbg("l_own", l_own)
        # ---------- tg chain (critical path to the first collective) ------
        wk_tg = load_w("wk_tg", wfull, 8)
        kp_tg = kvfull.tile([P, KO, E], BF16, tag="kv", name="kp_tg")
        gemm_fm(wk_tg, kvg, kp_tg, KO, act_copy=True)
        wv_tg = load_w("wv_tg", wfull, 8)
        vp_tg = kvfull.tile([P, KO, E], BF16, tag="kv", name="vp_tg")
        vproj_smajor(wv_tg, kvg, vp_tg, E)

        w_tg = load_w("w_tg", wfull, 8)
        t_g = acts.tile([P, KO, RG], BF16, tag="act", name="t_g")
        gemm_fm(w_tg, t_own, t_g, KO, bias=bias_pp["b_tg"])
        wq_tg = load_w("wq_tg", wfull, 8)
        qp_tg = qps.tile([P, KO, RG], BF16, tag="qp", name="qp_tg")
        gemm_fm(wq_tg, t_g, qp_tg, KO, bias=bias_pp["bq_tg"])

        _dbg("kp_tg", kp_tg)
        _dbg("vp_tg", vp_tg)
        _dbg("t_g", t_g)
        _dbg("qp_tg", qp_tg)
        ctx_tg = ctxs.tile([P, KO, RG], BF16, tag="cf", name="ctx_tg")
        attention(qp_tg,
                  lambda hc, s: kp_tg[:, hc, s * P:(s + 1) * P],
                  lambda s, cc: vp_tg[:, s, cc * P:(cc + 1) * P],
                  ctx_tg, bias_pp["bv_tg"], 4)
        wo_tg = load_w("wo_tg", wfull, 8)
        gt = acts.tile([P, KO, RG], BF16, tag="act", name="gt")
        gemm_fm(wo_tg, ctx_tg, gt, KO, bias=bias_pp["bo_tg"], residual=t_g)

        _dbg("ctx_tg", ctx_tg)
        _dbg("gt", gt)
        # ff K/V S-shard + quad AllGather (AG_C) -- issue ASAP
        outb_ff = kv_gather_issue("ff", gt)

        # ---------- tl chain (runs on the PE under AG_C) ----------
        wk_tl = load_w("wk_tl", wfull, 8)
        kp_tl = kvfull.tile([P, KO, E], BF16, tag="kv", name="kp_tl")
        gemm_fm(wk_tl, kvl, kp_tl, KO, act_copy=True)
        wv_tl = load_w("wv_tl", wfull, 8)
        vp_tl = kvfull.tile([P, KO, E], BF16, tag="kv", name="vp_tl")
        vproj_smajor(wv_tl, kvl, vp_tl, E)

        w_tl = load_w("w_tl", wfull, 8)
        t_l = acts.tile([P, KO, RG], BF16, tag="act", name="t_l")
        gemm_fm(w_tl, t_own, t_l, KO, bias=bias_pp["b_tl"])
        wq_tl = load_w("wq_tl", wfull, 8)
        qp_tl = qps.tile([P, KO, RG], BF16, tag="qp", name="qp_tl")
        gemm_fm(wq_tl, t_l, qp_tl, KO, bias=bias_pp["bq_tl"])

        ctx_tl = ctxs.tile([P, KO, RG], BF16, tag="cf", name="ctx_tl")
        attention(qp_tl,
                  lambda hc, s: kp_tl[:, hc, s * P:(s + 1) * P],
                  lambda s, cc: vp_tl[:, s, cc * P:(cc + 1) * P],
                  ctx_tl, bias_pp["bv_tl"], 4)
        wo_tl = load_w("wo_tl", wfull, 8)
        lt = acts.tile([P, KO, RG], BF16, tag="act", name="lt")
        gemm_fm(wo_tl, ctx_tl, lt, KO, bias=bias_pp["bo_tl"], residual=t_l)
        wq_ff = load_w("wq_ff", whalf, 4)
        qp_ff = qps.tile([P, 4, RG], BF16, tag="qph", name="qp_ff")
        gemm_fm(wq_ff, lt, qp_ff, 4, bias=bias_pp["bq_ff"])

        _dbg("t_l", t_l)
        _dbg("lt", lt)
        _dbg("qp_ff", qp_ff)
        # ---------- ff attention (waits on AG_C) ----------
        kp_ff_sl, vp_ff_sl = kv_gather_unpack("ff", outb_ff)
        ctxh_ff = ctxs.tile([P, 4, RG], BF16, tag="ch", name="ctxh_ff")
        attention(qp_ff, kp_ff_sl, vp_ff_sl, ctxh_ff, bias_pp["bv_ff"], 2)
        _dbg("ctxh_ff", ctxh_ff)
        outb_cff = ctx_gather_issue("ff", ctxh_ff)      # AG_D (pair)

        # t_r (full rows) + qp_rt fill the AG_D window
        xt = load_stream("x_text")
        bc_t = bcast_row(inv_t, E)
        for ko in range(KO):
            nc.vector.tensor_mul(xt[:, ko], xt[:, ko], bc_t)
        w_rep = load_w("w_rep", wfull, 8)
        t_r = pers.tile([P, KO, E], BF16, name="t_r")
        gemm_fm(w_rep, xt, t_r, KO, bias=bias_pp["b_rep"])
        wq_rt = load_w("wq_rt", whalf, 4)
        t_r_own = acts.tile([P, KO, RG], BF16, tag="act", name="t_r_own")
        gemm_fm(w_rep, t_own, t_r_own, KO, bias=bias_pp["b_rep"])
        qp_rt = qps.tile([P, 4, RG], BF16, tag="qph", name="qp_rt")
        gemm_fm(wq_rt, t_r_own, qp_rt, 4, bias=bias_pp["bq_rt"])

        # ff out-projection + residual -> ff activation
        ctxf_ff = ctx_gather_unpack("ff", outb_cff)
        wo_ff = load_w("wo_ff", wfull, 8)
        ffa = acts.tile([P, KO, RG], BF16, tag="act", name="ffa")
        gemm_fm(wo_ff, ctxf_ff, ffa, KO, bias=bias_pp["bo_ff"], residual=lt)

        _dbg("ctxf_ff", ctxf_ff)
        _dbg("ffa", ffa)
        _dbg("t_r", t_r)
        _dbg("qp_rt", qp_rt)
        # rt K/V S-shard + quad AllGather (AG_E)
        outb_rt = kv_gather_issue("rt", ffa)

        # ---------- rt attention (waits on AG_E) ----------
        kp_rt_sl, vp_rt_sl = kv_gather_unpack("rt", outb_rt)
        ctxh_rt = ctxs.tile([P, 4, RG], BF16, tag="ch", name="ctxh_rt")
        attention(qp_rt, kp_rt_sl, vp_rt_sl, ctxh_rt, bias_pp["bv_rt"], 2)
        outb_crt = ctx_gather_issue("rt", ctxh_rt)      # AG_F (pair)

        _dbg("ctxh_rt", ctxh_rt)
        ctxf_rt = ctx_gather_unpack("rt", outb_crt)
        wo_rt = load_w("wo_rt", wfull, 8)
        rt = acts.tile([P, KO, RG], BF16, tag="act", name="rt")
        gemm_fm(wo_rt, ctxf_rt, rt, KO, bias=bias_pp["bo_rt"])

        _dbg("rt_t", rt)
        # ---------- full = rt @ t_r.T, cosine logits ----------
        fullT = acts.tile([P, KO, RG], BF16, tag="act", name="fullT")
        for nchunk in range(KO):
            ps = ps256.tile([P, RG], F32, tag="mm", name="ps_full")
            for ko in range(KO):
                nc.tensor.matmul(ps, t_r[:, ko, nchunk * P:(nchunk + 1) * P],
                                 rt[:, ko], start=(ko == 0),
                                 stop=(ko == KO - 1))
            nc.vector.tensor_copy(fullT[:, nchunk], ps)

        inv_full = colsum_inv(fullT, KO, with_eps=True)
        bc_full = bcast_row(inv_full, RG)
        for ko in range(KO):
            nc.vector.tensor_mul(fullT[:, ko], fullT[:, ko], bc_full)

        lg = outs.tile([P, 2, RG], F32, name="lg")
        for lc in range(2):
            ps = ps256.tile([P, RG], F32, tag="mm", name="ps_lg")
            for ko in range(KO):
                nc.tensor.matmul(ps, fullT[:, ko, lc * P:(lc + 1) * P],
                                 l_own[:, ko], start=(ko == 0),
                                 stop=(ko == KO - 1))
            nc.vector.tensor_copy(lg[:, lc], ps)
        nc.sync.dma_start(out_logits.rearrange("(lc p) q -> p lc q", p=P), lg)

    nc.compile()
    return nc


def make_in_maps(local_feat, global_feat, text_feat,
                 w_tl, b_tl, w_tg, b_tg, w_rep, b_rep,
                 pos_local, pos_global, mha_params):
    """mha_params: dict m -> (wi, bi, wo, bo)."""
    f32 = np.float32
    bf16 = ml_dtypes.bfloat16

    def tb(x):
        return np.ascontiguousarray(np.asarray(x).T.astype(bf16))

    textT = tb(text_feat)
    locT = tb(local_feat)
    globT = tb(global_feat)
    shared = {
        "x_text": textT, "x_loc": locT, "x_glob": globT,
        "w_tl": tb(w_tl), "w_tg": tb(w_tg), "w_rep": tb(w_rep),
        "b_tl": b_tl.astype(f32), "b_tg": b_tg.astype(f32),
        "b_rep": b_rep.astype(f32),
        "pos_l": pos_local.astype(f32), "pos_g": pos_global.astype(f32),
    }
    for m in ("tl", "tg"):
        wi, bi, wo, bo = mha_params[m]
        shared[f"wq_{m}"] = tb(wi[0 * E:1 * E])
        shared[f"wk_{m}"] = tb(wi[1 * E:2 * E])
        shared[f"wv_{m}"] = tb(wi[2 * E:3 * E])
        shared[f"wo_{m}"] = tb(wo)
        shared[f"bq_{m}"] = bi[0 * E:1 * E].astype(f32)
        shared[f"bv_{m}"] = bi[2 * E:3 * E].astype(f32)
        shared[f"bo_{m}"] = bo.astype(f32)
    per_j = {}
    for j in range(2):
        d = {}
        sl = slice(512 * j, 512 * (j + 1))
        for m in ("ff", "rt"):
            wi, bi, wo, bo = mha_params[m]
            d[f"wq_{m}"] = tb(wi[0 * E:1 * E][sl])
            d[f"wk_{m}"] = tb(wi[1 * E:2 * E][sl])
            d[f"wv_{m}"] = tb(wi[2 * E:3 * E][sl])
            d[f"wo_{m}"] = tb(wo)
            d[f"bq_{m}"] = bi[0 * E:1 * E][sl].astype(f32)
            d[f"bv_{m}"] = bi[2 * E:3 * E][sl].astype(f32)
            d[f"bo_{m}"] = bo.astype(f32)
        per_j[j] = d

    in_maps = []
    for c in range(NCORES):
        g, j = c // 2, c % 2
        rs = slice(RG * g, RG * (g + 1))
        m = {
            "x_text_own": np.ascontiguousarray(textT[:, rs]),
            "x_loc_own": np.ascontiguousarray(locT[:, rs]),
        }
        m.update(shared)
        m.update(per_j[j])
        in_maps.append(m)
    return in_maps


def kernel(local_feat, global_feat, text_feat,
           w_tl, b_tl, w_tg, b_tg, w_rep, b_rep,
           pos_local, pos_global,
           tl_wi, tl_bi, tl_wo, tl_bo,
           tg_wi, tg_bi, tg_wo, tg_bo,
           ff_wi, ff_bi, ff_wo, ff_bo,
           rt_wi, rt_bi, rt_wo, rt_bo,
           n_groups):
    assert int(n_groups) == 4
    if "nc" not in _CACHE:
        _CACHE["nc"] = build_nc()
    nc = _CACHE["nc"]
    mha_params = {
        "tl": (tl_wi, tl_bi, tl_wo, tl_bo),
        "tg": (tg_wi, tg_bi, tg_wo, tg_bo),
        "ff": (ff_wi, ff_bi, ff_wo, ff_bo),
        "rt": (rt_wi, rt_bi, rt_wo, rt_bo),
    }
    in_maps = make_in_maps(np.asarray(local_feat), np.asarray(global_feat),
                           np.asarray(text_feat),
                           np.asarray(w_tl), np.asarray(b_tl),
                           np.asarray(w_tg), np.asarray(b_tg),
                           np.asarray(w_rep), np.asarray(b_rep),
                           np.asarray(pos_local), np.asarray(pos_global),
                           {k: tuple(np.asarray(x) for x in v)
                            for k, v in mha_params.items()})
    res = run_bass_kernel_spmd(nc, in_maps, core_ids=list(range(NCORES)))
    _CACHE["last_results"] = res
    out = np.empty((4, RG, RG), dtype=np.float32)
    for g in range(4):
        out[g] = res.results[2 * g]["logits"]
    return out


# revision 19
# speedup vs baseline: 1.9380x; 1.0376x over previous
"""Trainium2 Bass kernel for nn_Model4 (retrieval_knn).

Model: 3 l2-normalized feature streams -> 4 chained MultiheadAttention blocks
-> full = rt @ t_r.T -> per-group cosine logits [4, 256, 256].

Sharding (8 cores = 4 row-groups x 2 head-halves): core c = 2*g + j.
g owns rows R_g = [256g, 256g+256) (== final group g); j owns qkv feature
columns [512j, 512j+512) for the ff/rt MHAs only.

The collective cost (15us constant + bytes/40GBps, serialized on one device)
dominates, so the design minimizes collectives:
  - tl/tg MHAs: fully replicated within the pair (all 4 heads; K/V computed
    locally over the full sequence from the replicated local/global streams).
    No collective at all for these two MHAs.
  - t_r: computed in full on every core (a cheap GEMM beats a gather).
  - ff/rt MHAs: head-half sharded; K/V projections S-sharded then quad
    AllGather (bf16); context halves pair AllGather (bf16).
=> 4 collectives total vs the previous 6.

All GEMMs bf16 x bf16 with fp32 PSUM accumulation; the K-projection bias is
dropped (softmax is invariant to per-query constant score shifts); V bias is
added to the context after attention (attention weights sum to 1).  The
Activation engine uses only {square, ln, exp, copy} (one act table set ->
one table load); 1/sqrt(x) is computed as exp(-0.5*ln(x)).

Engine queues: SP = input/weight loads + output store; DVE = scales, bias
adds, context muls, collective pack DMAs and (deferred) unpack DMAs;
Act = squares/exp/ln + PSUM->SBUF copies of K/V; Pool = collectives and a
few adds.  Collective unpack DMAs are issued late in program order so they
never head-of-line block work that should run under the AllGather.
"""
import os
import sys

sys.path.insert(0, "/opt/trn_rl_repo")

import ml_dtypes
import numpy as np

import concourse.bass as bass  # noqa: F401
import concourse.tile as tile
import concourse.mybir as mybir
from concourse import bacc
from concourse.bass_utils import run_bass_kernel_spmd

E = 1024
P = 128
KO = E // P          # 8 feature chunks
RG = 256             # rows per group
NCORES = 8
F32 = mybir.dt.float32
F32R = mybir.dt.float32r
BF16 = mybir.dt.bfloat16
AF = mybir.ActivationFunctionType
GROUPS4 = [[0, 2, 4, 6], [1, 3, 5, 7]]   # gather S-shards across row-groups
GROUPS2 = [[0, 1], [2, 3], [4, 5], [6, 7]]  # exchange head halves within pair
EPS = 1e-8
KV_ELEMS = 512 * RG          # bf16 elems of one kp or vp piece
CTX_ELEMS = 512 * RG         # bf16 elems of one ctx piece

_CACHE = {}


def build_nc():
    nc = bacc.Bacc("TRN2", target_bir_lowering=False, debug=False,
                   num_devices=NCORES)
    dram = {}

    def din(name, shape, dt=BF16):
        dram[name] = nc.dram_tensor(name, shape, dt, kind="ExternalInput").ap()

    # full feature streams (feat-major: [feat, row]) + own-row slices
    din("x_text", [E, E])
    din("x_loc", [E, E])
    din("x_glob", [E, E])
    din("x_text_own", [E, RG])
    din("x_loc_own", [E, RG])
    for w in ("w_tl", "w_tg", "w_rep"):
        din(w, [E, E])
    for b in ("b_tl", "b_tg", "b_rep"):
        din(b, [E], F32)
    din("pos_l", [E], F32)
    din("pos_g", [E], F32)
    # tl/tg: full-head q/k/v weights; ff/rt: this core's head-half
    for m in ("tl", "tg"):
        din(f"wq_{m}", [E, E])
        din(f"wk_{m}", [E, E])
        din(f"wv_{m}", [E, E])
        din(f"wo_{m}", [E, E])
        din(f"bq_{m}", [E], F32)
        din(f"bv_{m}", [E], F32)
        din(f"bo_{m}", [E], F32)
    for m in ("ff", "rt"):
        din(f"wq_{m}", [E, 512])
        din(f"wk_{m}", [E, 512])
        din(f"wv_{m}", [E, 512])
        din(f"wo_{m}", [E, E])
        din(f"bq_{m}", [512], F32)
        din(f"bv_{m}", [512], F32)
        din(f"bo_{m}", [E], F32)

    out_logits = nc.dram_tensor("logits", [RG, RG], F32,
                                kind="ExternalOutput").ap()
    dbg_names = [x for x in os.environ.get("KDEBUG", "").split(",") if x]
    dbg_outs = {}

    from contextlib import ExitStack
    with tile.TileContext(nc) as tc, ExitStack() as ctx:
        consts = ctx.enter_context(tc.tile_pool(name="consts", bufs=1))
        streams = ctx.enter_context(tc.tile_pool(name="streams", bufs=2))
        kvfull = ctx.enter_context(tc.tile_pool(name="kvfull", bufs=2))
        wfull = ctx.enter_context(tc.tile_pool(name="wfull", bufs=2))
        whalf = ctx.enter_context(tc.tile_pool(name="whalf", bufs=2))
        acts = ctx.enter_context(tc.tile_pool(name="acts", bufs=4))
        pers = ctx.enter_context(tc.tile_pool(name="pers", bufs=1))
        qps = ctx.enter_context(tc.tile_pool(name="qps", bufs=2))
        exps = ctx.enter_context(tc.tile_pool(name="exps", bufs=1))
        sqs = ctx.enter_context(tc.tile_pool(name="sqs", bufs=2))
        ctxs = ctx.enter_context(tc.tile_pool(name="ctxs", bufs=1))
        bcs = ctx.enter_context(tc.tile_pool(name="bcs", bufs=1))
        smalls = ctx.enter_context(tc.tile_pool(name="smalls", bufs=1))
        outs = ctx.enter_context(tc.tile_pool(name="outs", bufs=1))
        ps512 = ctx.enter_context(tc.tile_pool(name="ps512", bufs=2,
                                               space="PSUM"))
        ps256 = ctx.enter_context(tc.tile_pool(name="ps256", bufs=3,
                                               space="PSUM"))
        pssum = ctx.enter_context(tc.tile_pool(name="pssum", bufs=1,
                                               space="PSUM"))
        dram_p = ctx.enter_context(tc.tile_pool(name="dram_p", bufs=1,
                                                space="DRAM"))

        # ---------- constants ----------
        ones_cb = consts.tile([P, 1], BF16)
        nc.vector.memset(ones_cb, 1.0)
        ones_cf = consts.tile([P, 1], F32)
        nc.vector.memset(ones_cf, 1.0)
        ones_col = consts.tile([P, 1], F32R)
        nc.vector.tensor_copy(ones_col, ones_cf)
        ones_row_f = consts.tile([1, P], F32)
        nc.vector.memset(ones_row_f, 1.0)
        ones_row = consts.tile([1, P], F32R)
        nc.vector.tensor_copy(ones_row, ones_row_f)

        def load_bias_pp(name, n):
            """[n] dram -> [128, n//128] per-partition scalar layout."""
            t = consts.tile([P, n // P], F32, name=f"c_{name}")
            nc.sync.dma_start(t, dram[name].rearrange("(c p) -> p c", p=P))
            return t

        # input streams first: they gate the first compute on the chain
        def load_stream(name):
            t = streams.tile([P, KO, E], BF16, tag="x", name=name)
            nc.sync.dma_start(t, dram[name].rearrange("(ko p) r -> p ko r",
                                                      p=P))
            return t

        kvg = load_stream("x_glob")            # becomes kvg in place
        kvl = load_stream("x_loc")             # becomes kvl in place
        t_own = pers.tile([P, KO, RG], BF16, name="textn_own")
        nc.sync.dma_start(t_own, dram["x_text_own"].rearrange(
            "(ko p) r -> p ko r", p=P))
        l_own = pers.tile([P, KO, RG], BF16, name="localn_own")
        nc.sync.dma_start(l_own, dram["x_loc_own"].rearrange(
            "(ko p) r -> p ko r", p=P))

        bias_pp = {}
        for nm in ("b_tl", "b_tg", "b_rep", "pos_l", "pos_g"):
            bias_pp[nm] = load_bias_pp(nm, E)
        for m in ("tl", "tg"):
            for bn in ("bq", "bv", "bo"):
                bias_pp[f"{bn}_{m}"] = load_bias_pp(f"{bn}_{m}", E)
        for m in ("ff", "rt"):
            for bn in ("bq", "bv"):
                bias_pp[f"{bn}_{m}"] = load_bias_pp(f"{bn}_{m}", 512)
            bias_pp[f"bo_{m}"] = load_bias_pp(f"bo_{m}", E)

        # ---------- debug ----------
        def _dbg(nm, t):
            if nm not in dbg_names:
                return
            do = nc.dram_tensor(f"dbg_{nm}", [P] + list(t.shape[1:]),
                                t.dtype, kind="ExternalOutput").ap()
            nc.sync.dma_start(do, t)

        # ---------- helpers ----------
        def load_w(name, pool, nco):
            """weight [1024, nco*128] dram -> [128, 8, nco*128] bf16 tile."""
            t = pool.tile([P, KO, nco * P], BF16, tag="w", name=f"w_{name}")
            nc.sync.dma_start(t, dram[name].rearrange("(ko p) c -> p ko c",
                                                      p=P))
            return t

        def colsum_inv(src, nko, with_eps=False):
            """src [128, nko, R] bf16: per-free-column 1/||col||, [1,R] f32r."""
            R = src.shape[2]
            inv = smalls.tile([1, R], F32R, tag=f"inv{R}", name="inv")
            for h in range(0, R, 512):
                w = min(512, R - h)
                ps = pssum.tile([1, w], F32, tag=f"cs{w}", name="ps_cs")
                for ko in range(nko):
                    sq = sqs.tile([P, w], F32R, tag=f"sq{w}", name="sq")
                    nc.scalar.activation(sq, src[:, ko, h:h + w], AF.Square)
                    nc.tensor.matmul(ps, ones_col, sq, start=(ko == 0),
                                     stop=(ko == nko - 1))
                src_ps = ps
                if with_eps:
                    mx = smalls.tile([1, w], F32, tag=f"mx{w}", name="mx")
                    nc.vector.tensor_scalar_max(mx, ps, EPS * EPS)
                    src_ps = mx
                rec = smalls.tile([1, w], F32, tag=f"rc{w}", name="rec")
                with nc.allow_low_precision(reason="fp32 reciprocal+sqrt"):
                    nc.vector.reciprocal(rec, src_ps)
                nc.scalar.activation(inv[:, h:h + w], rec, AF.Sqrt)
            return inv

        def bcast_row(row_f32r, n):
            """[1, n] f32r -> [128, n] f32 broadcast via K=1 outer product."""
            out = bcs.tile([P, n], F32, tag=f"bc{n}", name="bc")
            for h in range(0, n, 512):
                w = min(512, n - h)
                pool, tag = (ps256, "mm") if w <= 256 else (ps512, "mm512")
                ps = pool.tile([P, w], F32, tag=tag, name="ps_bc")
                nc.tensor.matmul(ps, ones_row, row_f32r[:, h:h + w],
                                 start=True, stop=True)
                nc.vector.tensor_copy(out[:, h:h + w], ps)
            return out

        def gemm_fm(w_sb, act, out, nco, bias=None, residual=None,
                    eng=None, act_copy=False):
            """Feat-major GEMM: out[:, c, :] = w[:, :, c*128:].T @ act (+b+r).
            w_sb [128, 8, nco*128] bf16; act [128, 8, R] bf16;
            out [128, nco, R]; bias [128, >=nco] f32; residual bf16."""
            R = act.shape[2]
            eng = eng or nc.vector
            for c in range(nco):
                for h in range(0, R, 512):
                    w = min(512, R - h)
                    pool, tag = (ps256, "mm") if w <= 256 else (ps512, "mm512")
                    ps = pool.tile([P, w], F32, tag=tag, name="ps_g")
                    for ko in range(KO):
                        nc.tensor.matmul(ps, w_sb[:, ko, c * P:(c + 1) * P],
                                         act[:, ko, h:h + w], start=(ko == 0),
                                         stop=(ko == KO - 1))
                    o = out[:, c, h:h + w]
                    if bias is not None:
                        eng.tensor_scalar_add(o, ps, bias[:, c:c + 1])
                        if residual is not None:
                            nc.gpsimd.tensor_add(o, o, residual[:, c, h:h + w])
                    elif residual is not None:
                        eng.tensor_scalar_add(o, ps, 0.0)
                        nc.gpsimd.tensor_add(o, o, residual[:, c, h:h + w])
                    elif act_copy:
                        nc.scalar.copy(o, ps)
                    else:
                        eng.tensor_copy(o, ps)

        def vproj_smajor(w_sb, act, vp, nchan):
            """S-major V projection: vp[:, s, :] = act[:, :, s128].T @ w.
            act [128, 8, S] bf16 feat-major; w_sb [128, 8, nchan];
            vp [128, S//128, nchan] bf16 (no bias: bv folds into ctx)."""
            S = act.shape[2]
            for s in range(S // P):
                for h in range(0, nchan, 512):
                    w = min(512, nchan - h)
                    ps = ps512.tile([P, w], F32, tag="mm512", name="ps_v")
                    for ko in range(KO):
                        nc.tensor.matmul(ps, act[:, ko, s * P:(s + 1) * P],
                                         w_sb[:, ko, h:h + w],
                                         start=(ko == 0), stop=(ko == KO - 1))
                    nc.scalar.copy(vp[:, s, h:h + w], ps)

        def attention(qp, kp_sl, vp_sl, ctx_out, bv_pp, nheads):
            """qp [128,2*nheads,256] bf16 feat-major.
            kp_sl(hc, s) -> [128,128] bf16 lhsT (proj-chan chunk hc, S chunk s)
            vp_sl(s, cc) -> [128,128] bf16 lhsT (S chunk s, ctx chunk cc).
            ctx_out [128,2*nheads,256] bf16.  d=256 per head (2 chunks)."""
            ns = 8
            for h in range(nheads):
                expt = exps.tile([P, ns, RG], BF16, tag="exp", name="expt")
                pss = pssum.tile([1, RG], F32, tag="cs256", name="ps_sm")
                for s in range(ns):
                    ps = ps256.tile([P, RG], F32, tag="mm", name="ps_sc")
                    for dk in range(2):
                        nc.tensor.matmul(ps, kp_sl(2 * h + dk, s),
                                         qp[:, 2 * h + dk],
                                         start=(dk == 0), stop=(dk == 1))
                    nc.scalar.activation(expt[:, s], ps, AF.Exp, scale=0.0625)
                for s in range(ns):
                    nc.tensor.matmul(pss, ones_cb, expt[:, s],
                                     start=(s == 0), stop=(s == ns - 1))
                inv = smalls.tile([1, RG], F32R, tag="invsm", name="inv_sm")
                with nc.allow_low_precision(reason="fp32r rounding intended"):
                    nc.vector.reciprocal(inv, pss)
                bc = bcast_row(inv, RG)
                for dk in range(2):
                    cc = 2 * h + dk
                    ps = ps256.tile([P, RG], F32, tag="mm", name="ps_av")
                    for s in range(ns):
                        nc.tensor.matmul(ps, vp_sl(s, cc), expt[:, s],
                                         start=(s == 0), stop=(s == ns - 1))
                    nc.vector.tensor_mul(ctx_out[:, cc], ps, bc)
                    nc.gpsimd.tensor_scalar_add(ctx_out[:, cc],
                                                ctx_out[:, cc],
                                                bv_pp[:, cc:cc + 1])

        def pack_piece(inbuf, off, sb_tile):
            """bf16 SBUF tile -> bf16 dram flat buffer."""
            shp = sb_tile.shape
            n = P * shp[1] * shp[2]
            nc.sync.dma_start(
                inbuf[off:off + n].rearrange("(p a b) -> p a b", p=P,
                                             a=shp[1]), sb_tile)

        def allgather(inbuf, outbuf, groups):
            nc.gpsimd.collective_compute(
                "AllGather", mybir.AluOpType.bypass,
                replica_groups=groups,
                ins=[inbuf.opt()], outs=[outbuf.opt()])

        def kv_gather_issue(m, kv_src):
            """S-shard k/v projection for this core's head-half + quad AG."""
            wk = load_w(f"wk_{m}", whalf, 4)
            kp = ctxs.tile([P, 4, RG], BF16, tag="kp", name=f"kp_{m}")
            gemm_fm(wk, kv_src, kp, 4, act_copy=True)
            wv = load_w(f"wv_{m}", whalf, 4)
            vp = ctxs.tile([P, 2, 512], BF16, tag="vp", name=f"vp_{m}")
            vproj_smajor(wv, kv_src, vp, 512)
            inb = dram_p.tile([2 * KV_ELEMS], BF16, name=f"in_{m}")
            outb = dram_p.tile([4, 2 * KV_ELEMS], BF16, name=f"out_{m}")
            _dbg(f"kp_{m}_piece", kp)
            _dbg(f"vp_{m}_piece", vp)
            pack_piece(inb, 0, kp)
            pack_piece(inb, KV_ELEMS, vp)
            if f"packback_{m}" in dbg_names:
                t_pb = consts.tile([P, 4, RG], BF16, name=f"t_pb_{m}")
                nc.sync.dma_start(
                    t_pb,
                    inb[0:KV_ELEMS].rearrange("(p a b) -> p a b", p=P, a=4))
                _dbg(f"packback_{m}", t_pb)
            allgather(inb, outb, GROUPS4)
            return outb

        def kv_gather_unpack(m, outb):
            """Deferred unpack of the quad-AG result into full-S k/v tiles."""
            kpf = kvfull.tile([P, 4, 4, RG], BF16, tag="kv", name=f"kpf_{m}",
                              padded_shape=[P, 4, 4, 2 * RG])
            vpf = kvfull.tile([P, 4, 2, 512], BF16, tag="kv", name=f"vpf_{m}",
                              padded_shape=[P, 4, 2, 1024])
            for gs in range(4):
                nc.sync.dma_start(
                    kpf[:, gs],
                    outb[gs, 0:KV_ELEMS].rearrange("(p a b) -> p a b",
                                                   p=P, a=4))
                nc.sync.dma_start(
                    vpf[:, gs],
                    outb[gs, KV_ELEMS:].rearrange("(p a b) -> p a b",
                                                  p=P, a=2))
            _dbg(f"kpf_{m}", kpf)
            _dbg(f"vpf_{m}", vpf)
            kp_sl = lambda hc, s: kpf[:, s // 2, hc, (s % 2) * P:(s % 2 + 1) * P]
            vp_sl = lambda s, cc: vpf[:, s // 2, s % 2, cc * P:(cc + 1) * P]
            return kp_sl, vp_sl

        def ctx_gather_issue(m, ctx_half):
            inb = dram_p.tile([CTX_ELEMS], BF16, name=f"inc_{m}")
            outb = dram_p.tile([2, CTX_ELEMS], BF16, name=f"outc_{m}")
            pack_piece(inb, 0, ctx_half)
            allgather(inb, outb, GROUPS2)
            return outb

        def ctx_gather_unpack(m, outb):
            full = ctxs.tile([P, KO, RG], BF16, tag="cf", name=f"ctxf_{m}")
            for r in range(2):
                nc.sync.dma_start(
                    full[:, 4 * r:4 * r + 4],
                    outb[r].rearrange("(p a b) -> p a b", p=P, a=4))
            return full

        # ---------- stage 0: normalize ----------
        # glob first (feeds the tg chain = critical path)
        inv_g = colsum_inv(kvg, KO)
        bc_g = bcast_row(inv_g, E)
        for ko in range(KO):
            nc.vector.tensor_mul(kvg[:, ko], kvg[:, ko], bc_g)
            nc.gpsimd.tensor_scalar_add(kvg[:, ko], kvg[:, ko],
                                        bias_pp["pos_g"][:, ko:ko + 1])
        # local stream + own-row slices (off the critical path engines-wise)
        inv_l = colsum_inv(kvl, KO)
        bc_l = bcast_row(inv_l, E)
        for ko in range(KO):
            nc.vector.tensor_mul(kvl[:, ko], kvl[:, ko], bc_l)
            nc.gpsimd.tensor_scalar_add(kvl[:, ko], kvl[:, ko],
                                        bias_pp["pos_l"][:, ko:ko + 1])
        inv_to = colsum_inv(t_own, KO)
        bc_to = bcast_row(inv_to, RG)
        for ko in range(KO):
            nc.vector.tensor_mul(t_own[:, ko], t_own[:, ko], bc_to)
        inv_lo = colsum_inv(l_own, KO)
        bc_lo = bcast_row(inv_lo, RG)
        for ko in range(KO):
            nc.vector.tensor_mul(l_own[:, ko], l_own[:, ko], bc_lo)
        # full-text norms, computed early via chunked DMA so every Rsqrt
        # happens before the first Exp (act tables: one switch each way)
        inv_t = smalls.tile([1, E], F32R, tag="inv1024", name="inv_t")
        for h in range(2):
            ps = pssum.tile([1, 512], F32, tag="cs512", name="ps_xt")
            for ko in range(KO):
                stg = sqs.tile([P, 512], BF16, tag="stg", name="stg")
                nc.sync.dma_start(
                    stg, dram["x_text"][ko * P:(ko + 1) * P,
                                        h * 512:(h + 1) * 512])
                sq = sqs.tile([P, 512], F32R, tag="sq512", name="sq")
                nc.scalar.activation(sq, stg, AF.Square)
                nc.tensor.matmul(ps, ones_col, sq, start=(ko == 0),
                                 stop=(ko == KO - 1))
            rec = smalls.tile([1, 512], F32, tag="rc512", name="rec_t")
            with nc.allow_low_precision(reason="fp32 reciprocal+sqrt"):
                nc.vector.reciprocal(rec, ps)
            nc.scalar.activation(inv_t[:, h * 512:(h + 1) * 512], rec,
                                 AF.Sqrt)

        _dbg("kvg", kvg)
        _dbg("kvl", kvl)
        _dbg("t_own", t_own)
        _dbg("l_own", l_own)
        # ---------- tg chain (critical path to the first collective) ------
        wk_tg = load_w("wk_tg", wfull, 8)
        kp_tg = kvfull.tile([P, KO, E], BF16, tag="kv", name="kp_tg")
        gemm_fm(wk_tg, kvg, kp_tg, KO, act_copy=True)
        wv_tg = load_w("wv_tg", wfull, 8)
        vp_tg = kvfull.tile([P, KO, E], BF16, tag="kv", name="vp_tg")
        vproj_smajor(wv_tg, kvg, vp_tg, E)

        w_tg = load_w("w_tg", wfull, 8)
        t_g = acts.tile([P, KO, RG], BF16, tag="act", name="t_g")
        gemm_fm(w_tg, t_own, t_g, KO, bias=bias_pp["b_tg"])
        wq_tg = load_w("wq_tg", wfull, 8)
        qp_tg = qps.tile([P, KO, RG], BF16, tag="qp", name="qp_tg")
        gemm_fm(wq_tg, t_g, qp_tg, KO, bias=bias_pp["bq_tg"])

        _dbg("kp_tg", kp_tg)
        _dbg("vp_tg", vp_tg)
        _dbg("t_g", t_g)
        _dbg("qp_tg", qp_tg)
        ctx_tg = ctxs.tile([P, KO, RG], BF16, tag="cf", name="ctx_tg")
        attention(qp_tg,
                  lambda hc, s: kp_tg[:, hc, s * P:(s + 1) * P],
                  lambda s, cc: vp_tg[:, s, cc * P:(cc + 1) * P],
                  ctx_tg, bias_pp["bv_tg"], 4)
        wo_tg = load_w("wo_tg", wfull, 8)
        gt = acts.tile([P, KO, RG], BF16, tag="act", name="gt")
        gemm_fm(wo_tg, ctx_tg, gt, KO, bias=bias_pp["bo_tg"], residual=t_g)

        _dbg("ctx_tg", ctx_tg)
        _dbg("gt", gt)
        # ff K/V S-shard + quad AllGather (AG_C) -- issue ASAP
        outb_ff = kv_gather_issue("ff", gt)

        # ---------- tl chain (runs on the PE under AG_C) ----------
        wk_tl = load_w("wk_tl", wfull, 8)
        kp_tl = kvfull.tile([P, KO, E], BF16, tag="kv", name="kp_tl")
        gemm_fm(wk_tl, kvl, kp_tl, KO, act_copy=True)
        wv_tl = load_w("wv_tl", wfull, 8)
        vp_tl = kvfull.tile([P, KO, E], BF16, tag="kv", name="vp_tl")
        vproj_smajor(wv_tl, kvl, vp_tl, E)

        w_tl = load_w("w_tl", wfull, 8)
        t_l = acts.tile([P, KO, RG], BF16, tag="act", name="t_l")
        gemm_fm(w_tl, t_own, t_l, KO, bias=bias_pp["b_tl"])
        wq_tl = load_w("wq_tl", wfull, 8)
        qp_tl = qps.tile([P, KO, RG], BF16, tag="qp", name="qp_tl")
        gemm_fm(wq_tl, t_l, qp_tl, KO, bias=bias_pp["bq_tl"])

        ctx_tl = ctxs.tile([P, KO, RG], BF16, tag="cf", name="ctx_tl")
        attention(qp_tl,
                  lambda hc, s: kp_tl[:, hc, s * P:(s + 1) * P],
                  lambda s, cc: vp_tl[:, s, cc * P:(cc + 1) * P],
                  ctx_tl, bias_pp["bv_tl"], 4)
        wo_tl = load_w("wo_tl", wfull, 8)
        lt = acts.tile([P, KO, RG], BF16, tag="act", name="lt")
        gemm_fm(wo_tl, ctx_tl, lt, KO, bias=bias_pp["bo_tl"], residual=t_l)
        wq_ff = load_w("wq_ff", whalf, 4)
        qp_ff = qps.tile([P, 4, RG], BF16, tag="qph", name="qp_ff")
        gemm_fm(wq_ff, lt, qp_ff, 4, bias=bias_pp["bq_ff"])

        _dbg("t_l", t_l)
        _dbg("lt", lt)
        _dbg("qp_ff", qp_ff)
        # ---------- ff attention (waits on AG_C) ----------
        kp_ff_sl, vp_ff_sl = kv_gather_unpack("ff", outb_ff)
        ctxh_ff = ctxs.tile([P, 4, RG], BF16, tag="ch", name="ctxh_ff")
        attention(qp_ff, kp_ff_sl, vp_ff_sl, ctxh_ff, bias_pp["bv_ff"], 2)
        _dbg("ctxh_ff", ctxh_ff)
        outb_cff = ctx_gather_issue("ff", ctxh_ff)      # AG_D (pair)

        # t_r (full rows) + qp_rt fill the AG_D window
        xt = load_stream("x_text")
        bc_t = bcast_row(inv_t, E)
        for ko in range(KO):
            nc.vector.tensor_mul(xt[:, ko], xt[:, ko], bc_t)
        w_rep = load_w("w_rep", wfull, 8)
        t_r = pers.tile([P, KO, E], BF16, name="t_r")
        gemm_fm(w_rep, xt, t_r, KO, bias=bias_pp["b_rep"])
        wq_rt = load_w("wq_rt", whalf, 4)
        t_r_own = acts.tile([P, KO, RG], BF16, tag="act", name="t_r_own")
        gemm_fm(w_rep, t_own, t_r_own, KO, bias=bias_pp["b_rep"])
        qp_rt = qps.tile([P, 4, RG], BF16, tag="qph", name="qp_rt")
        gemm_fm(wq_rt, t_r_own, qp_rt, 4, bias=bias_pp["bq_rt"])

        # ff out-projection + residual -> ff activation
        ctxf_ff = ctx_gather_unpack("ff", outb_cff)
        wo_ff = load_w("wo_ff", wfull, 8)
        ffa = acts.tile([P, KO, RG], BF16, tag="act", name="ffa")
        gemm_fm(wo_ff, ctxf_ff, ffa, KO, bias=bias_pp["bo_ff"], residual=lt)

        _dbg("ctxf_ff", ctxf_ff)
        _dbg("ffa", ffa)
        _dbg("t_r", t_r)
        _dbg("qp_rt", qp_rt)
        # rt K/V S-shard + quad AllGather (AG_E)
        outb_rt = kv_gather_issue("rt", ffa)

        # B = wo_rt . t_r and g = t_r.T bo_rt fill the AG_E window, so the
        # post-AG_F tail needs no rt out-projection: full.T = B.T@ctx_rt + g
        wo_rt_fm = load_w("wo_rt", wfull, 8)
        Bm = streams.tile([P, KO, E], BF16, tag="x", name="Bmat")
        gemm_fm(wo_rt_fm, t_r, Bm, KO)
        bo_rt_bf = consts.tile([P, KO], BF16, name="bo_rt_bf")
        nc.vector.tensor_copy(bo_rt_bf, bias_pp["bo_rt"])
        g_pp = consts.tile([P, KO], F32, name="g_pp")
        for nch in range(KO):
            psg = pssum.tile([P, 1], F32, tag="g1", name="ps_g1")
            for ko in range(KO):
                nc.tensor.matmul(psg, t_r[:, ko, nch * P:(nch + 1) * P],
                                 bo_rt_bf[:, ko:ko + 1], start=(ko == 0),
                                 stop=(ko == KO - 1))
            nc.vector.tensor_copy(g_pp[:, nch:nch + 1], psg)

        # ---------- rt attention (waits on AG_E) ----------
        kp_rt_sl, vp_rt_sl = kv_gather_unpack("rt", outb_rt)
        ctxh_rt = ctxs.tile([P, 4, RG], BF16, tag="ch", name="ctxh_rt")
        attention(qp_rt, kp_rt_sl, vp_rt_sl, ctxh_rt, bias_pp["bv_rt"], 2)
        outb_crt = ctx_gather_issue("rt", ctxh_rt)      # AG_F (pair)

        _dbg("ctxh_rt", ctxh_rt)
        ctxf_rt = ctx_gather_unpack("rt", outb_crt)

        # ---------- full.T = B.T @ ctx_rt + g, cosine logits ----------
        fullT = acts.tile([P, KO, RG], BF16, tag="act", name="fullT")
        for nchunk in range(KO):
            ps = ps256.tile([P, RG], F32, tag="mm", name="ps_full")
            for cc in range(KO):
                nc.tensor.matmul(ps, Bm[:, cc, nchunk * P:(nchunk + 1) * P],
                                 ctxf_rt[:, cc], start=(cc == 0),
                                 stop=(cc == KO - 1))
            nc.vector.tensor_scalar_add(fullT[:, nchunk], ps,
                                        g_pp[:, nchunk:nchunk + 1])

        inv_full = colsum_inv(fullT, KO, with_eps=True)
        bc_full = bcast_row(inv_full, RG)
        for ko in range(KO):
            nc.vector.tensor_mul(fullT[:, ko], fullT[:, ko], bc_full)

        lg = outs.tile([P, 2, RG], F32, name="lg")
        for lc in range(2):
            ps = ps256.tile([P, RG], F32, tag="mm", name="ps_lg")
            for ko in range(KO):
                nc.tensor.matmul(ps, fullT[:, ko, lc * P:(lc + 1) * P],
                                 l_own[:, ko], start=(ko == 0),
                                 stop=(ko == KO - 1))
            nc.vector.tensor_copy(lg[:, lc], ps)
        nc.sync.dma_start(out_logits.rearrange("(lc p) q -> p lc q", p=P), lg)

    nc.compile()
    return nc


def make_in_maps(local_feat, global_feat, text_feat,
                 w_tl, b_tl, w_tg, b_tg, w_rep, b_rep,
                 pos_local, pos_global, mha_params):
    """mha_params: dict m -> (wi, bi, wo, bo)."""
    f32 = np.float32
    bf16 = ml_dtypes.bfloat16

    def tb(x):
        return np.ascontiguousarray(np.asarray(x).T.astype(bf16))

    textT = tb(text_feat)
    locT = tb(local_feat)
    globT = tb(global_feat)
    shared = {
        "x_text": textT, "x_loc": locT, "x_glob": globT,
        "w_tl": tb(w_tl), "w_tg": tb(w_tg), "w_rep": tb(w_rep),
        "b_tl": b_tl.astype(f32), "b_tg": b_tg.astype(f32),
        "b_rep": b_rep.astype(f32),
        "pos_l": pos_local.astype(f32), "pos_g": pos_global.astype(f32),
    }
    for m in ("tl", "tg"):
        wi, bi, wo, bo = mha_params[m]
        shared[f"wq_{m}"] = tb(wi[0 * E:1 * E])
        shared[f"wk_{m}"] = tb(wi[1 * E:2 * E])
        shared[f"wv_{m}"] = tb(wi[2 * E:3 * E])
        shared[f"wo_{m}"] = tb(wo)
        shared[f"bq_{m}"] = bi[0 * E:1 * E].astype(f32)
        shared[f"bv_{m}"] = bi[2 * E:3 * E].astype(f32)
        shared[f"bo_{m}"] = bo.astype(f32)
    per_j = {}
    for j in range(2):
        d = {}
        sl = slice(512 * j, 512 * (j + 1))
        for m in ("ff", "rt"):
            wi, bi, wo, bo = mha_params[m]
            d[f"wq_{m}"] = tb(wi[0 * E:1 * E][sl])
            d[f"wk_{m}"] = tb(wi[1 * E:2 * E][sl])
            d[f"wv_{m}"] = tb(wi[2 * E:3 * E][sl])
            # wo_rt is consumed as B = wo_rt . t_r (contraction over the
            # output-feature axis), so it ships untransposed
            d[f"wo_{m}"] = (np.ascontiguousarray(np.asarray(wo).astype(
                ml_dtypes.bfloat16)) if m == "rt" else tb(wo))
            d[f"bq_{m}"] = bi[0 * E:1 * E][sl].astype(f32)
            d[f"bv_{m}"] = bi[2 * E:3 * E][sl].astype(f32)
            d[f"bo_{m}"] = bo.astype(f32)
        per_j[j] = d

    in_maps = []
    for c in range(NCORES):
        g, j = c // 2, c % 2
        rs = slice(RG * g, RG * (g + 1))
        m = {
            "x_text_own": np.ascontiguousarray(textT[:, rs]),
            "x_loc_own": np.ascontiguousarray(locT[:, rs]),
        }
        m.update(shared)
        m.update(per_j[j])
        in_maps.append(m)
    return in_maps


def kernel(local_feat, global_feat, text_feat,
           w_tl, b_tl, w_tg, b_tg, w_rep, b_rep,
           pos_local, pos_global,
           tl_wi, tl_bi, tl_wo, tl_bo,
           tg_wi, tg_bi, tg_wo, tg_bo,
           ff_wi, ff_bi, ff_wo, ff_bo,
           rt_wi, rt_bi, rt_wo, rt_bo,
           n_groups):
    assert int(n_groups) == 4
    if "nc" not in _CACHE:
        _CACHE["nc"] = build_nc()
    nc = _CACHE["nc"]
    mha_params = {
        "tl": (tl_wi, tl_bi, tl_wo, tl_bo),
        "tg": (tg_wi, tg_bi, tg_wo, tg_bo),
        "ff": (ff_wi, ff_bi, ff_wo, ff_bo),
        "rt": (rt_wi, rt_bi, rt_wo, rt_bo),
    }
    in_maps = make_in_maps(np.asarray(local_feat), np.asarray(global_feat),
                           np.asarray(text_feat),
                           np.asarray(w_tl), np.asarray(b_tl),
                           np.asarray(w_tg), np.asarray(b_tg),
                           np.asarray(w_rep), np.asarray(b_rep),
                           np.asarray(pos_local), np.asarray(pos_global),
                           {k: tuple(np.asarray(x) for x in v)
                            for k, v in mha_params.items()})
    res = run_bass_kernel_spmd(nc, in_maps, core_ids=list(range(NCORES)))
    _CACHE["last_results"] = res
    out = np.empty((4, RG, RG), dtype=np.float32)
    for g in range(4):
        out[g] = res.results[2 * g]["logits"]
    return out


# revision 59
# speedup vs baseline: 2.1807x; 1.1252x over previous
"""Trainium2 Bass kernel for nn_Model4 (retrieval_knn).

Model: 3 l2-normalized feature streams -> 4 chained MultiheadAttention blocks
-> full = rt @ t_r.T -> per-group cosine logits [4, 256, 256].

Sharding (8 cores = 4 row-groups x 2 head-halves): core c = 2*g + j.
g owns rows R_g = [256g, 256g+256) (== final group g); j owns qkv feature
columns [512j, 512j+512) for the ff/rt MHAs only.

The collective cost (15us constant + bytes/40GBps, serialized on one device)
dominates, so the design minimizes collectives:
  - tl/tg MHAs: fully replicated within the pair (all 4 heads; K/V computed
    locally over the full sequence).  No collective at all for these two.
  - t_r computed in full on every core; B = wo_rt . t_r and g = t_r.T bo_rt
    precomputed during the AG_E window so the tail is full.T = B.T@ctx + g.
  - ff/rt MHAs: head-half sharded; K/V S-sharded + quad AllGather; context
    halves pair AllGather.  => 4 collectives total.

Mixed precision: fp8e4 (x16 scaling) with DoubleRow matmuls on every path
whose error is damped before the logits (K projections and scores
everywhere; V/context for tl/tg/ff, whose outputs are ~3% of the residual
stream; all wire payloads except ctx_rt).  bf16 is kept on the
logit-critical chain: t_l -> lt -> ff -> V_rt -> ctx_rt -> full, plus t_r,
B, and the final cosine stage.  fp32 PSUM accumulation throughout.

Scale conventions: fp8 weights and activations carry x16; K/V projection
psums therefore carry x256 and are stored as-is; scores carry x4096 (folded
into the exp scale); fp8 AV contexts are rescaled to x16 via a 1/16-valued
broadcast; biases feeding fp8 tensors ship pre-scaled x16 from the host.
The K-projection bias is dropped entirely (softmax is invariant to
per-query constant score shifts); V bias is added to the context after
attention (attention weights sum to 1).
"""
import os
import sys

sys.path.insert(0, "/opt/trn_rl_repo")

import ml_dtypes
import numpy as np

import concourse.bass as bass  # noqa: F401
import concourse.tile as tile
import concourse.mybir as mybir
from concourse import bacc
from concourse.bass_utils import run_bass_kernel_spmd

E = 1024
P = 128
KO = E // P          # 8 feature chunks
RG = 256             # rows per group
NCORES = 8
F32 = mybir.dt.float32
F32R = mybir.dt.float32r
BF16 = mybir.dt.bfloat16
FP8 = mybir.dt.float8e4
U8 = mybir.dt.uint8
DRow = mybir.MatmulPerfMode.DoubleRow
AF = mybir.ActivationFunctionType
GROUPS4 = [[0, 2, 4, 6], [1, 3, 5, 7]]   # gather S-shards across row-groups
GROUPS2 = [[0, 1], [2, 3], [4, 5], [6, 7]]  # exchange head halves within pair
EPS = 1e-8
S16 = 16.0                   # fp8 scale
SC_EXP = 0.0625 / (S16 * S16 * S16 * S16)   # d^-0.5 / (qp x16 . kp x256)
KV8 = 512 * RG               # bytes of one fp8 kp or vp piece
CTXB = 512 * RG * 2          # bytes of one bf16 ctx piece (rt)
VPB = 512 * RG * 2           # bytes of one bf16 vp piece (rt)

_CACHE = {}


def build_nc():
    nc = bacc.Bacc("TRN2", target_bir_lowering=False, debug=False,
                   num_devices=NCORES)
    dram = {}

    def din(name, shape, dt=BF16):
        dram[name] = nc.dram_tensor(name, shape, dt, kind="ExternalInput").ap()

    din("x_text", [E, E])
    din("x_loc", [E, E])
    din("x_glob", [E, E])
    din("x_loc8", [E, E], FP8)   # pre-scaled x16, raw (K path)
    din("x_glob8", [E, E], FP8)  # pre-scaled x16, raw (K path)
    din("x_text_own", [E, RG])
    din("x_loc_own", [E, RG])
    for w in ("w_tl", "w_tg", "w_rep"):
        din(w, [E, E])
    for b in ("b_tl", "b_tg", "b_rep"):
        din(b, [E], F32)
    # Q/K weights fp8 x16 (scores tolerate it); V/out weights bf16.
    # pos embeddings are folded on the host: the K-side term is softmax-
    # invariant (dropped), the V-side term folds into bv.
    for m in ("tl", "tg"):
        din(f"wq_{m}", [E, E], FP8)
        din(f"wk_{m}", [E, E], FP8)
        din(f"wv_{m}", [E, E])
        din(f"wo_{m}", [E, E])
        din(f"bq_{m}", [E], F32)     # x256
        din(f"bo_{m}", [E], F32)     # x1; bv (incl. pos fold) via wo
    for m in ("ff", "rt"):
        din(f"wq_{m}", [E, 512], FP8)
        din(f"wk_{m}", [E, 512], FP8)
        din(f"wv_{m}", [E, 512])
        din(f"wo_{m}", [E, E])
        din(f"bq_{m}", [512], F32)   # x256
        din(f"bo_{m}", [E], F32)     # x1; bv folded via wo
    out_logits = nc.dram_tensor("logits", [RG, RG], F32,
                                kind="ExternalOutput").ap()
    dbg_names = [x for x in os.environ.get("KDEBUG", "").split(",") if x]

    from contextlib import ExitStack
    with tile.TileContext(nc) as tc, ExitStack() as ctx:
        consts = ctx.enter_context(tc.tile_pool(name="consts", bufs=1))
        streams = ctx.enter_context(tc.tile_pool(name="streams", bufs=2))
        kv8s = ctx.enter_context(tc.tile_pool(name="kv8s", bufs=1))
        kvfull = ctx.enter_context(tc.tile_pool(name="kvfull", bufs=1))
        kvbig = ctx.enter_context(tc.tile_pool(name="kvbig", bufs=1))
        kvbf = ctx.enter_context(tc.tile_pool(name="kvbf", bufs=1))
        wfull = ctx.enter_context(tc.tile_pool(name="wfull", bufs=3))
        wfull8 = ctx.enter_context(tc.tile_pool(name="wfull8", bufs=1))
        whalf8 = ctx.enter_context(tc.tile_pool(name="whalf8", bufs=2))
        acts = ctx.enter_context(tc.tile_pool(name="acts", bufs=4))
        acts8 = ctx.enter_context(tc.tile_pool(name="acts8", bufs=2))
        pers = ctx.enter_context(tc.tile_pool(name="pers", bufs=1))
        qps = ctx.enter_context(tc.tile_pool(name="qps", bufs=1))
        exps = ctx.enter_context(tc.tile_pool(name="exps", bufs=2))
        sqs = ctx.enter_context(tc.tile_pool(name="sqs", bufs=2))
        ctxs = ctx.enter_context(tc.tile_pool(name="ctxs", bufs=1))
        bcs = ctx.enter_context(tc.tile_pool(name="bcs", bufs=1))
        smalls = ctx.enter_context(tc.tile_pool(name="smalls", bufs=1))
        ps512 = ctx.enter_context(tc.tile_pool(name="ps512", bufs=3,
                                               space="PSUM"))
        ps256 = ctx.enter_context(tc.tile_pool(name="ps256", bufs=3,
                                               space="PSUM"))
        pssum = ctx.enter_context(tc.tile_pool(name="pssum", bufs=1,
                                               space="PSUM"))
        dram_p = ctx.enter_context(tc.tile_pool(name="dram_p", bufs=1,
                                                space="DRAM"))

        # ---------- constants ----------
        ones_cb = consts.tile([P, 1], BF16)
        nc.vector.memset(ones_cb, 1.0)
        ones_cf = consts.tile([P, 1], F32)
        nc.vector.memset(ones_cf, 1.0)
        ones_col = consts.tile([P, 1], F32R)
        nc.vector.tensor_copy(ones_col, ones_cf)

        def row_const(val):
            tf = consts.tile([1, P], F32, name=f"rc_{val}")
            nc.vector.memset(tf, val)
            tr = consts.tile([1, P], F32R, name=f"rcr_{val}")
            nc.vector.tensor_copy(tr, tf)
            return tr

        ones_row = row_const(1.0)

        def load_bias_pp(name, n):
            t = consts.tile([P, n // P], F32, name=f"c_{name}")
            nc.sync.dma_start(t, dram[name].rearrange("(c p) -> p c", p=P))
            return t

        # ---------- input streams first (gate the first compute) ----------
        def load_stream(name):
            t = streams.tile([P, KO, E], BF16, tag="x", name=name)
            nc.sync.dma_start(t, dram[name].rearrange("(ko p) r -> p ko r",
                                                      p=P))
            return t

        kvg = load_stream("x_glob")            # becomes globn in place
        kvl = load_stream("x_loc")             # becomes localn in place
        kvg8 = kv8s.tile([P, KO, E], FP8, tag="kv8", name="kvg8")
        nc.sync.dma_start(kvg8, dram["x_glob8"].rearrange(
            "(ko p) r -> p ko r", p=P))
        t_own = pers.tile([P, KO, RG], BF16, name="textn_own")
        nc.sync.dma_start(t_own, dram["x_text_own"].rearrange(
            "(ko p) r -> p ko r", p=P))
        l_own = pers.tile([P, KO, RG], BF16, name="localn_own")
        nc.sync.dma_start(l_own, dram["x_loc_own"].rearrange(
            "(ko p) r -> p ko r", p=P))

        bias_pp = {}
        for nm in ("b_tl", "b_tg", "b_rep"):
            bias_pp[nm] = load_bias_pp(nm, E)
        for m in ("tl", "tg"):
            bias_pp[f"bq_{m}"] = load_bias_pp(f"bq_{m}", E)
            bias_pp[f"bo_{m}"] = load_bias_pp(f"bo_{m}", E)
        for m in ("ff", "rt"):
            bias_pp[f"bq_{m}"] = load_bias_pp(f"bq_{m}", 512)
            bias_pp[f"bo_{m}"] = load_bias_pp(f"bo_{m}", E)

        # ---------- debug ----------
        def _dbg(nm, t):
            if nm not in dbg_names:
                return
            do = nc.dram_tensor(f"dbg_{nm}", [P] + list(t.shape[1:]),
                                t.dtype, kind="ExternalOutput").ap()
            nc.sync.dma_start(do, t)

        # ---------- helpers ----------
        def load_w(name, pool, nco, dt=BF16, pad=None):
            t = pool.tile([P, KO, nco * P], dt, tag="w", name=f"w_{name}",
                          padded_shape=[P, KO, pad * P] if pad else None)
            nc.sync.dma_start(t, dram[name].rearrange("(ko p) c -> p ko c",
                                                      p=P))
            return t

        def colsum_inv(src, nko, with_eps=False):
            """src [128, nko, R] bf16: per-free-column 1/||col||, [1,R] f32r."""
            R = src.shape[2]
            inv = smalls.tile([1, R], F32R, tag=f"inv{R}", name="inv")
            for h in range(0, R, 512):
                w = min(512, R - h)
                ps = pssum.tile([1, w], F32, tag=f"cs{w}", name="ps_cs")
                for ko in range(nko):
                    sq = sqs.tile([P, w], BF16, tag=f"sq{w}", name="sq")
                    nc.scalar.activation(sq, src[:, ko, h:h + w], AF.Square)
                    nc.tensor.matmul(ps, ones_cb, sq, start=(ko == 0),
                                     stop=(ko == nko - 1))
                rec = smalls.tile([1, w], F32, tag=f"rc{w}", name="rec")
                with nc.allow_low_precision(reason="fp32 recip+sqrt"):
                    nc.vector.reciprocal(rec, ps)
                if with_eps:
                    # 1/max(ss, eps^2) == min(1/ss, eps^-2), incl. ss == 0
                    nc.vector.tensor_scalar_min(rec, rec,
                                                1.0 / (EPS * EPS))
                nc.scalar.activation(inv[:, h:h + w], rec, AF.Sqrt)
            return inv

        def bcast_row(row_f32r, n, ones=None):
            """[1, n] f32r -> [128, n] f32 broadcast scaled by the ones val."""
            out = bcs.tile([P, n], F32, tag=f"bc{n}", name="bc")
            for h in range(0, n, 512):
                w = min(512, n - h)
                pool, tag = (ps256, "mm") if w <= 256 else (ps512, "mm512")
                ps = pool.tile([P, w], F32, tag=tag, name="ps_bc")
                nc.tensor.matmul(ps, ones or ones_row, row_f32r[:, h:h + w],
                                 start=True, stop=True)
                nc.vector.tensor_copy(out[:, h:h + w], ps)
            return out

        ADD = mybir.AluOpType.add
        MULT = mybir.AluOpType.mult

        def psum_out(o, ps, bias=None, scale=None, residual=None,
                     act_copy=False):
            """PSUM -> SBUF with optional (x scale), (+ bias), (+ residual)."""
            if bias is not None and residual is not None:
                assert scale is None
                nc.vector.scalar_tensor_tensor(o, ps, bias, residual,
                                               ADD, ADD)
            elif bias is not None and scale is not None:
                # (ps + bias) * scale -- biases ship pre-multiplied so that
                # this fused form lands in the target fp8/bf16 scale
                nc.vector.tensor_scalar(o, ps, bias, scale, ADD, MULT)
            elif bias is not None:
                nc.vector.tensor_scalar_add(o, ps, bias)
            elif scale is not None:
                nc.vector.tensor_scalar_mul(o, ps, scale)
            elif act_copy:
                nc.scalar.copy(o, ps)
            else:
                nc.vector.tensor_copy(o, ps)
            if residual is not None and bias is None:
                nc.gpsimd.tensor_add(o, o, residual)

        def gemm_fm(w_sb, act, out, nco, bias=None, scale=None,
                    residual=None, act_copy=False):
            """bf16 feat-major GEMM: out[:,c,:] = w[:,:,c128].T @ act."""
            R = act.shape[2]
            for c in range(nco):
                for h in range(0, R, 512):
                    w = min(512, R - h)
                    pool, tag = (ps256, "mm") if w <= 256 else (ps512, "mm512")
                    ps = pool.tile([P, w], F32, tag=tag, name="ps_g")
                    for ko in range(KO):
                        nc.tensor.matmul(ps, w_sb[:, ko, c * P:(c + 1) * P],
                                         act[:, ko, h:h + w], start=(ko == 0),
                                         stop=(ko == KO - 1))
                    psum_out(out[:, c, h:h + w], ps,
                             bias[:, c:c + 1] if bias is not None else None,
                             scale,
                             residual[:, c, h:h + w] if residual is not None
                             else None, act_copy)

        def gemm_dr(w8, act8, out, nco, bias=None, scale=None,
                    residual=None, act_copy=False, colscale=None):
            """fp8 DoubleRow GEMM (4x PE): same contract as gemm_fm.
            colscale: [128, R] broadcast tile multiplied per output column
            (commuted norm scaling for the K projections)."""
            R = act8.shape[2]
            for c in range(nco):
                for h in range(0, R, 512):
                    w = min(512, R - h)
                    pool, tag = (ps256, "mm") if w <= 256 else (ps512, "mm512")
                    ps = pool.tile([P, w], F32, tag=tag, name="ps_g")
                    for k2 in range(0, KO, 2):
                        nc.tensor.matmul(ps,
                                         w8[:, k2:k2 + 2, c * P:(c + 1) * P],
                                         act8[:, k2:k2 + 2, h:h + w],
                                         start=(k2 == 0), stop=(k2 == KO - 2),
                                         perf_mode=DRow)
                    if colscale is not None:
                        nc.vector.tensor_mul(out[:, c, h:h + w], ps,
                                             colscale[:, h:h + w])
                        continue
                    psum_out(out[:, c, h:h + w], ps,
                             bias[:, c:c + 1] if bias is not None else None,
                             scale,
                             residual[:, c, h:h + w] if residual is not None
                             else None, act_copy)

        def vproj_smajor(w_sb, act, vp, nchan):
            """bf16 S-major V projection: vp[:,s,:] = act[:,:,s128].T @ w."""
            S = act.shape[2]
            for s in range(S // P):
                for h in range(0, nchan, 512):
                    w = min(512, nchan - h)
                    ps = ps512.tile([P, w], F32, tag="mm512", name="ps_v")
                    for ko in range(KO):
                        nc.tensor.matmul(ps, act[:, ko, s * P:(s + 1) * P],
                                         w_sb[:, ko, h:h + w],
                                         start=(ko == 0), stop=(ko == KO - 1))
                    nc.scalar.copy(vp[:, s, h:h + w], ps)

        def attention_mx(qp8, kp2, vp_sl, ctx_out, nheads):
            """fp8 DoubleRow scores + bf16 denominator/AV/context.
            Heads run in interleaved pairs so the cross-engine exp/recip/
            broadcast latencies hide under the other head's matmuls; the
            pair shares one reciprocal and one broadcast.  The V bias is
            folded into the out-projection bias on the host."""
            for hp in range(0, nheads, 2):
                pss = pssum.tile([1, 2 * RG], F32, tag="cs512", name="ps_sm")
                expts = []
                for i in range(2):
                    h = hp + i
                    expt = exps.tile([P, KO, RG], BF16, tag="expb",
                                     name=f"exptb{i}")
                    for s in range(KO):
                        ps = ps256.tile([P, RG], F32, tag="mm", name="ps_sc")
                        nc.tensor.matmul(ps, kp2(h, s),
                                         qp8[:, 2 * h:2 * h + 2],
                                         start=True, stop=True,
                                         perf_mode=DRow)
                        nc.scalar.activation(expt[:, s], ps, AF.Exp,
                                             scale=SC_EXP)
                    expts.append(expt)
                for i in range(2):
                    for s in range(KO):
                        nc.tensor.matmul(pss[:, i * RG:(i + 1) * RG],
                                         ones_cb, expts[i][:, s],
                                         start=(s == 0), stop=(s == KO - 1))
                inv = smalls.tile([1, 2 * RG], F32R, tag="invsm",
                                  name="inv_sm")
                with nc.allow_low_precision(reason="fp32r rounding intended"):
                    nc.vector.reciprocal(inv, pss)
                bc = bcast_row(inv, 2 * RG)
                for i in range(2):
                    h = hp + i
                    for dk in range(2):
                        cc = 2 * h + dk
                        ps = ps256.tile([P, RG], F32, tag="mm", name="ps_av")
                        for s in range(KO):
                            nc.tensor.matmul(ps, vp_sl(s, cc),
                                             expts[i][:, s],
                                             start=(s == 0),
                                             stop=(s == KO - 1))
                        nc.vector.tensor_mul(ctx_out[:, cc], ps,
                                             bc[:, i * RG:(i + 1) * RG])

        def pack_piece(inbuf, off, sb_tile):
            """SBUF tile -> byte-typed dram flat buffer (uint8 bitcast)."""
            t = sb_tile.bitcast(U8)
            shp = t.shape
            n = P * shp[1] * shp[2]
            nc.sync.dma_start(
                inbuf[off:off + n].rearrange("(p a b) -> p a b", p=P,
                                             a=shp[1]), t)

        def allgather(inbuf, outbuf, groups):
            nc.gpsimd.collective_compute(
                "AllGather", mybir.AluOpType.bypass,
                replica_groups=groups,
                ins=[inbuf.opt()], outs=[outbuf.opt()])

        # ---------- stage 0: normalize ----------
        # glob first (feeds the tg chain = critical path).  The bf16 master
        # is normalized in place (V path); the raw fp8 copy feeds the K
        # projection, whose column norm scaling commutes to the kp output.
        inv_g = colsum_inv(kvg, KO)
        bc_g = bcast_row(inv_g, E)
        for ko in range(KO):
            nc.vector.tensor_mul(kvg[:, ko], kvg[:, ko], bc_g)
        inv_l = colsum_inv(kvl, KO)
        bc_l = bcast_row(inv_l, E)
        for ko in range(KO):
            nc.vector.tensor_mul(kvl[:, ko], kvl[:, ko], bc_l)
        inv_to = colsum_inv(t_own, KO)
        bc_to = bcast_row(inv_to, RG)
        for ko in range(KO):
            nc.vector.tensor_mul(t_own[:, ko], t_own[:, ko], bc_to)
        inv_lo = colsum_inv(l_own, KO)
        bc_lo = bcast_row(inv_lo, RG)
        for ko in range(KO):
            nc.vector.tensor_mul(l_own[:, ko], l_own[:, ko], bc_lo)
        # full-text norms via chunked DMA (all Sqrt before the first Exp)
        inv_t = smalls.tile([1, E], F32R, tag="inv1024", name="inv_t")
        for h in range(2):
            ps = pssum.tile([1, 512], F32, tag="cs512", name="ps_xt")
            for ko in range(KO):
                stg = sqs.tile([P, 512], BF16, tag="stg", name="stg")
                nc.sync.dma_start(
                    stg, dram["x_text"][ko * P:(ko + 1) * P,
                                        h * 512:(h + 1) * 512])
                sq = sqs.tile([P, 512], BF16, tag="sq512", name="sq")
                nc.scalar.activation(sq, stg, AF.Square)
                nc.tensor.matmul(ps, ones_cb, sq, start=(ko == 0),
                                 stop=(ko == KO - 1))
            rec = smalls.tile([1, 512], F32, tag="rc512", name="rec_t")
            with nc.allow_low_precision(reason="fp32 recip+sqrt"):
                nc.vector.reciprocal(rec, ps)
            nc.scalar.activation(inv_t[:, h * 512:(h + 1) * 512], rec,
                                 AF.Sqrt)

        _dbg("kvg8", kvg8)
        _dbg("t_own", t_own)
        _dbg("l_own", l_own)

        # ---------- tg chain (critical path to the first collective) ------
        wk_tg = load_w("wk_tg", wfull8, 8, FP8)
        kp_tg = kvfull.tile([P, KO, E], FP8, tag="kp8", name="kp_tg")
        gemm_dr(wk_tg, kvg8, kp_tg, KO, colscale=bc_g)
        wv_tg = load_w("wv_tg", wfull, 8)
        vp_tg = kvbig.tile([P, KO, E], BF16, tag="vpb", name="vp_tg")
        vproj_smajor(wv_tg, kvg, vp_tg, E)

        w_tg = load_w("w_tg", wfull, 8)
        t_g = acts.tile([P, KO, RG], BF16, tag="act", name="t_g")
        gemm_fm(w_tg, t_own, t_g, KO, bias=bias_pp["b_tg"])
        t_g8 = acts8.tile([P, KO, RG], FP8, tag="act8", name="t_g8")
        for ko in range(KO):
            nc.scalar.activation(t_g8[:, ko], t_g[:, ko], AF.Copy, scale=S16)
        wq_tg = load_w("wq_tg", wfull8, 8, FP8)
        qp_tg = qps.tile([P, KO, RG], FP8, tag="qp", name="qp_tg")
        gemm_dr(wq_tg, t_g8, qp_tg, KO, bias=bias_pp["bq_tg"],
                scale=1.0 / S16)

        ctx_tg = ctxs.tile([P, KO, RG], BF16, tag="cf", name="ctx_tg")
        attention_mx(qp_tg,
                     lambda h, s: kp_tg[:, 2 * h:2 * h + 2,
                                        s * P:(s + 1) * P],
                     lambda s, cc: vp_tg[:, s, cc * P:(cc + 1) * P],
                     ctx_tg, 4)
        wo_tg = load_w("wo_tg", wfull, 8)
        gt = acts.tile([P, KO, RG], BF16, tag="act", name="gt")
        gemm_fm(wo_tg, ctx_tg, gt, KO, bias=bias_pp["bo_tg"], residual=t_g)
        gt8 = acts8.tile([P, KO, RG], FP8, tag="act8", name="gt8")
        for ko in range(KO):
            nc.scalar.activation(gt8[:, ko], gt[:, ko], AF.Copy, scale=S16)

        # ff K/V S-shard + quad AllGather (AG_C) -- issue ASAP
        wk_ff = load_w("wk_ff", whalf8, 4, FP8)
        kp_ff = ctxs.tile([P, 4, RG], FP8, tag="kp", name="kp_ff")
        gemm_dr(wk_ff, gt8, kp_ff, 4, act_copy=True)
        wv_ff = load_w("wv_ff", wfull, 4, pad=8)
        vp_ff = ctxs.tile([P, 2, 512], BF16, tag="vp", name="vp_ff")
        vproj_smajor(wv_ff, gt, vp_ff, 512)
        _dbg("kp_ff_piece", kp_ff)
        _dbg("vp_ff_piece", vp_ff)
        in_ff = dram_p.tile([KV8 + VPB], U8, name="in_ff")
        out_ff = dram_p.tile([4, KV8 + VPB], U8, name="out_ff")
        pack_piece(in_ff, 0, kp_ff)
        pack_piece(in_ff, KV8, vp_ff)
        allgather(in_ff, out_ff, GROUPS4)

        # ---------- tl chain (runs on the PE under AG_C) ----------
        # kvl8 shares kvg8's slab; its DMA is issued only here so the SP
        # queue can't head-of-line block the tg-chain weight loads on it
        kvl8 = kv8s.tile([P, KO, E], FP8, tag="kv8", name="kvl8")
        nc.sync.dma_start(kvl8, dram["x_loc8"].rearrange(
            "(ko p) r -> p ko r", p=P))
        wk_tl = load_w("wk_tl", wfull8, 8, FP8)
        kp_tl = kvfull.tile([P, KO, E], FP8, tag="kp8", name="kp_tl")
        gemm_dr(wk_tl, kvl8, kp_tl, KO, colscale=bc_l)
        wv_tl = load_w("wv_tl", wfull, 8)
        vp_tl = kvbig.tile([P, KO, E], BF16, tag="vpb", name="vp_tl")
        vproj_smajor(wv_tl, kvl, vp_tl, E)

        w_tl = load_w("w_tl", wfull, 8)
        t_l = acts.tile([P, KO, RG], BF16, tag="act", name="t_l")
        gemm_fm(w_tl, t_own, t_l, KO, bias=bias_pp["b_tl"])
        t_l8 = acts8.tile([P, KO, RG], FP8, tag="act8", name="t_l8")
        for ko in range(KO):
            nc.scalar.activation(t_l8[:, ko], t_l[:, ko], AF.Copy, scale=S16)
        wq_tl = load_w("wq_tl", wfull8, 8, FP8)
        qp_tl = qps.tile([P, KO, RG], FP8, tag="qp", name="qp_tl")
        gemm_dr(wq_tl, t_l8, qp_tl, KO, bias=bias_pp["bq_tl"],
                scale=1.0 / S16)

        ctx_tl = ctxs.tile([P, KO, RG], BF16, tag="cf", name="ctx_tl")
        attention_mx(qp_tl,
                     lambda h, s: kp_tl[:, 2 * h:2 * h + 2,
                                        s * P:(s + 1) * P],
                     lambda s, cc: vp_tl[:, s, cc * P:(cc + 1) * P],
                     ctx_tl, 4)
        wo_tl = load_w("wo_tl", wfull, 8)
        lt = acts.tile([P, KO, RG], BF16, tag="act", name="lt")
        gemm_fm(wo_tl, ctx_tl, lt, KO, bias=bias_pp["bo_tl"], residual=t_l)
        lt8 = acts8.tile([P, KO, RG], FP8, tag="act8", name="lt8")
        for ko in range(KO):
            nc.scalar.activation(lt8[:, ko], lt[:, ko], AF.Copy, scale=S16)
        wq_ff = load_w("wq_ff", whalf8, 4, FP8)
        qp_ff = qps.tile([P, 4, RG], FP8, tag="qph", name="qp_ff")
        gemm_dr(wq_ff, lt8, qp_ff, 4, bias=bias_pp["bq_ff"], scale=1.0 / S16)

        _dbg("t_l", t_l)
        _dbg("lt", lt)

        # ---------- ff attention (waits on AG_C) ----------
        kpf_ff = kvfull.tile([P, 4, 4, RG], FP8, tag="kp8", name="kpf_ff",
                             padded_shape=[P, 4, 4, 2 * RG])
        vpf_ff = kvbig.tile([P, 4, 2, 512], BF16, tag="vpb", name="vpf_ff",
                            padded_shape=[P, 4, 2, 1024])
        for gs in range(4):
            nc.sync.dma_start(
                kpf_ff[:, gs].bitcast(U8),
                out_ff[gs, 0:KV8].rearrange("(p a b) -> p a b", p=P, a=4))
            nc.sync.dma_start(
                vpf_ff[:, gs].bitcast(U8),
                out_ff[gs, KV8:].rearrange("(p a b) -> p a b", p=P, a=2))
        _dbg("kpf_ff", kpf_ff)
        ctxh_ff = ctxs.tile([P, 4, RG], BF16, tag="cf", name="ctxh_ff",
                            padded_shape=[P, 4, 2 * RG])
        attention_mx(qp_ff,
                     lambda h, s: kpf_ff[:, s // 2, 2 * h:2 * h + 2,
                                         (s % 2) * P:(s % 2 + 1) * P],
                     lambda s, cc: vpf_ff[:, s // 2, s % 2,
                                          cc * P:(cc + 1) * P],
                     ctxh_ff, 2)
        _dbg("ctxh_ff", ctxh_ff)
        in_cff = dram_p.tile([CTXB], U8, name="in_cff")
        out_cff = dram_p.tile([2, CTXB], U8, name="out_cff")
        pack_piece(in_cff, 0, ctxh_ff)
        allgather(in_cff, out_cff, GROUPS2)

        # t_r (full rows) + qp_rt fill the AG_D window
        xt = load_stream("x_text")
        bc_t = bcast_row(inv_t, E)
        for ko in range(KO):
            nc.vector.tensor_mul(xt[:, ko], xt[:, ko], bc_t)
        w_rep = load_w("w_rep", wfull, 8)
        t_r = kvbig.tile([P, KO, E], BF16, tag="vpb", name="t_r")
        gemm_fm(w_rep, xt, t_r, KO, bias=bias_pp["b_rep"])
        wq_rt = load_w("wq_rt", whalf8, 4, FP8)
        t_r_own8 = acts8.tile([P, KO, RG], FP8, tag="act8", name="t_r_own8")
        gemm_fm(w_rep, t_own, t_r_own8, KO, bias=bias_pp["b_rep"], scale=S16)
        qp_rt = qps.tile([P, 4, RG], FP8, tag="qph", name="qp_rt")
        gemm_dr(wq_rt, t_r_own8, qp_rt, 4, bias=bias_pp["bq_rt"],
                scale=1.0 / S16)

        # ff out-projection + residual -> ff activation (bf16 + fp8 copy)
        ctxf_ff = ctxs.tile([P, KO, RG], BF16, tag="cf", name="ctxf_ff")
        for r in range(2):
            nc.sync.dma_start(
                ctxf_ff[:, 4 * r:4 * r + 4].bitcast(U8),
                out_cff[r].rearrange("(p a b) -> p a b", p=P, a=4))
        wo_ff = load_w("wo_ff", wfull, 8)
        ffa = acts.tile([P, KO, RG], BF16, tag="act", name="ffa")
        gemm_fm(wo_ff, ctxf_ff, ffa, KO, bias=bias_pp["bo_ff"], residual=lt)
        ffa8 = acts8.tile([P, KO, RG], FP8, tag="act8", name="ffa8")
        for ko in range(KO):
            nc.scalar.activation(ffa8[:, ko], ffa[:, ko], AF.Copy, scale=S16)
        _dbg("ffa", ffa)

        # rt K/V S-shard + quad AllGather (AG_E): K fp8, V bf16
        wk_rt = load_w("wk_rt", whalf8, 4, FP8)
        kp_rt = ctxs.tile([P, 4, RG], FP8, tag="kp", name="kp_rt")
        gemm_dr(wk_rt, ffa8, kp_rt, 4, act_copy=True)
        wv_rt = load_w("wv_rt", wfull, 4, pad=8)
        vp_rt = kvbf.tile([P, 2, 512], BF16, tag="rtkv", name="vp_rt",
                          padded_shape=[P, 2, 2048])
        vproj_smajor(wv_rt, ffa, vp_rt, 512)
        in_rt = dram_p.tile([KV8 + VPB], U8, name="in_rt")
        out_rt = dram_p.tile([4, KV8 + VPB], U8, name="out_rt")
        pack_piece(in_rt, 0, kp_rt)
        pack_piece(in_rt, KV8, vp_rt)
        allgather(in_rt, out_rt, GROUPS4)

        # B = wo_rt . t_r and g = t_r.T bo_rt fill the AG_E window, so the
        # post-AG_F tail needs no rt out-projection: full.T = B.T@ctx_rt + g
        wo_rt_fm = load_w("wo_rt", wfull, 8)
        Bm = streams.tile([P, KO, E], BF16, tag="x", name="Bmat")
        gemm_fm(wo_rt_fm, t_r, Bm, KO)
        bo_rt_bf = consts.tile([P, KO], BF16, name="bo_rt_bf")
        nc.vector.tensor_copy(bo_rt_bf, bias_pp["bo_rt"])
        g_pp = consts.tile([P, KO], F32, name="g_pp")
        for nch in range(KO):
            psg = pssum.tile([P, 1], F32, tag="g1", name="ps_g1")
            for ko in range(KO):
                nc.tensor.matmul(psg, t_r[:, ko, nch * P:(nch + 1) * P],
                                 bo_rt_bf[:, ko:ko + 1], start=(ko == 0),
                                 stop=(ko == KO - 1))
            nc.vector.tensor_copy(g_pp[:, nch:nch + 1], psg)

        # ---------- rt attention (waits on AG_E) ----------
        kpf_rt = kvfull.tile([P, 4, 4, RG], FP8, tag="kp8", name="kpf_rt",
                             padded_shape=[P, 4, 4, 2 * RG])
        vpf_rt = kvbf.tile([P, 4, 2, 512], BF16, tag="rtkv",
                           name="vpf_rt")
        for gs in range(4):
            nc.sync.dma_start(
                kpf_rt[:, gs].bitcast(U8),
                out_rt[gs, 0:KV8].rearrange("(p a b) -> p a b", p=P, a=4))
            nc.sync.dma_start(
                vpf_rt[:, gs].bitcast(U8),
                out_rt[gs, KV8:].rearrange("(p a b) -> p a b", p=P, a=2))
        ctxh_rt = acts.tile([P, 4, RG], BF16, tag="act", name="ctxh_rt",
                            padded_shape=[P, 4, 2 * RG])
        attention_mx(qp_rt,
                     lambda h, s: kpf_rt[:, s // 2, 2 * h:2 * h + 2,
                                         (s % 2) * P:(s % 2 + 1) * P],
                     lambda s, cc: vpf_rt[:, s // 2, s % 2,
                                          cc * P:(cc + 1) * P],
                     ctxh_rt, 2)
        _dbg("ctxh_rt", ctxh_rt)
        in_crt = dram_p.tile([CTXB], U8, name="in_crt")
        out_crt = dram_p.tile([2, CTXB], U8, name="out_crt")
        pack_piece(in_crt, 0, ctxh_rt)
        allgather(in_crt, out_crt, GROUPS2)

        ctxf_rt = kvbf.tile([P, KO, RG], BF16, tag="rtkv", name="ctxf_rt",
                            padded_shape=[P, KO, 2 * RG])
        for r in range(2):
            nc.sync.dma_start(
                ctxf_rt[:, 4 * r:4 * r + 4].bitcast(U8),
                out_crt[r].rearrange("(p a b) -> p a b", p=P, a=4))

        # ---------- full.T = B.T @ ctx_rt + g, cosine logits ----------
        fullT = acts.tile([P, KO, RG], BF16, tag="act", name="fullT")
        for nchunk in range(KO):
            ps = ps256.tile([P, RG], F32, tag="mm", name="ps_full")
            for cc in range(KO):
                nc.tensor.matmul(ps, Bm[:, cc, nchunk * P:(nchunk + 1) * P],
                                 ctxf_rt[:, cc], start=(cc == 0),
                                 stop=(cc == KO - 1))
            nc.vector.tensor_scalar_add(fullT[:, nchunk], ps,
                                        g_pp[:, nchunk:nchunk + 1])

        inv_full = colsum_inv(fullT, KO, with_eps=True)
        # transpose the per-row inverse norms to a per-partition layout via
        # a tiny DRAM roundtrip, then fold the cosine normalization into the
        # logits copy-out as a per-partition scale
        invq_d = dram_p.tile([RG], F32R, name="invq_d")
        nc.sync.dma_start(invq_d, inv_full)
        invq = smalls.tile([P, 2], F32, tag="invq", name="invq")
        nc.sync.dma_start(invq, invq_d.rearrange("(c p) -> p c", p=P))

        lg = bcs.tile([P, 2, RG], F32, tag="bc1024", name="lg",
                      padded_shape=[P, 2, 2 * RG])
        for lc in range(2):
            ps = ps256.tile([P, RG], F32, tag="mm", name="ps_lg")
            for ko in range(KO):
                nc.tensor.matmul(ps, fullT[:, ko, lc * P:(lc + 1) * P],
                                 l_own[:, ko], start=(ko == 0),
                                 stop=(ko == KO - 1))
            nc.vector.tensor_scalar_mul(lg[:, lc], ps, invq[:, lc:lc + 1])
        nc.sync.dma_start(out_logits.rearrange("(lc p) q -> p lc q", p=P), lg)

    nc.compile()
    return nc


def make_in_maps(local_feat, global_feat, text_feat,
                 w_tl, b_tl, w_tg, b_tg, w_rep, b_rep,
                 pos_local, pos_global, mha_params):
    """mha_params: dict m -> (wi, bi, wo, bo)."""
    f32 = np.float32
    bf16 = ml_dtypes.bfloat16
    fp8 = ml_dtypes.float8_e4m3

    def tb(x):
        return np.ascontiguousarray(np.asarray(x).T.astype(bf16))

    def t8(x):
        return np.ascontiguousarray(
            (np.asarray(x).T.astype(f32) * 16.0).astype(fp8))

    textT = tb(text_feat)
    locT = tb(local_feat)
    shared = {
        "x_text": textT, "x_loc": locT, "x_glob": tb(global_feat),
        "x_loc8": t8(local_feat), "x_glob8": t8(global_feat),
        "w_tl": tb(w_tl), "w_tg": tb(w_tg), "w_rep": tb(w_rep),
        "b_tl": b_tl.astype(f32), "b_tg": b_tg.astype(f32),
        "b_rep": b_rep.astype(f32),
    }
    pos = {"tl": np.asarray(pos_local, dtype=np.float64),
           "tg": np.asarray(pos_global, dtype=np.float64)}
    for m in ("tl", "tg"):
        wi, bi, wo, bo = mha_params[m]
        shared[f"wq_{m}"] = t8(wi[0 * E:1 * E])
        shared[f"wk_{m}"] = t8(wi[1 * E:2 * E])
        shared[f"wv_{m}"] = tb(wi[2 * E:3 * E])
        shared[f"wo_{m}"] = tb(wo)
        shared[f"bq_{m}"] = bi[0 * E:1 * E].astype(f32) * 256.0
        # parameter-only folds: the V-side pos term joins bv, and the
        # whole V bias is pushed through the out-projection into bo; the
        # K-side pos term is softmax-invariant and dropped
        bv_fold = (np.asarray(bi[2 * E:3 * E], dtype=np.float64)
                   + pos[m] @ np.asarray(wi[2 * E:3 * E],
                                         dtype=np.float64).T)
        shared[f"bo_{m}"] = (np.asarray(bo, dtype=np.float64)
                             + bv_fold
                             @ np.asarray(wo, dtype=np.float64).T
                             ).astype(f32)
    per_j = {}
    for j in range(2):
        d = {}
        sl = slice(512 * j, 512 * (j + 1))
        for m in ("ff", "rt"):
            wi, bi, wo, bo = mha_params[m]
            d[f"wq_{m}"] = t8(wi[0 * E:1 * E][sl])
            d[f"wk_{m}"] = t8(wi[1 * E:2 * E][sl])
            d[f"wv_{m}"] = tb(wi[2 * E:3 * E][sl])
            if m == "rt":
                # wo_rt is consumed as B = wo_rt . t_r (contraction over
                # the output-feature axis), so it ships untransposed
                d[f"wo_{m}"] = np.ascontiguousarray(
                    np.asarray(wo).astype(bf16))
            else:
                d[f"wo_{m}"] = tb(wo)
            d[f"bq_{m}"] = bi[0 * E:1 * E][sl].astype(f32) * 256.0
            d[f"bo_{m}"] = (np.asarray(bo, dtype=np.float64)
                            + np.asarray(bi[2 * E:3 * E], dtype=np.float64)
                            @ np.asarray(wo, dtype=np.float64).T
                            ).astype(f32)
        per_j[j] = d

    in_maps = []
    for c in range(NCORES):
        g, j = c // 2, c % 2
        rs = slice(RG * g, RG * (g + 1))
        m = {
            "x_text_own": np.ascontiguousarray(textT[:, rs]),
            "x_loc_own": np.ascontiguousarray(locT[:, rs]),
        }
        m.update(shared)
        m.update(per_j[j])
        in_maps.append(m)
    return in_maps


def kernel(local_feat, global_feat, text_feat,
           w_tl, b_tl, w_tg, b_tg, w_rep, b_rep,
           pos_local, pos_global,
           tl_wi, tl_bi, tl_wo, tl_bo,
           tg_wi, tg_bi, tg_wo, tg_bo,
           ff_wi, ff_bi, ff_wo, ff_bo,
           rt_wi, rt_bi, rt_wo, rt_bo,
           n_groups):
    assert int(n_groups) == 4
    if "nc" not in _CACHE:
        _CACHE["nc"] = build_nc()
    nc = _CACHE["nc"]
    mha_params = {
        "tl": (tl_wi, tl_bi, tl_wo, tl_bo),
        "tg": (tg_wi, tg_bi, tg_wo, tg_bo),
        "ff": (ff_wi, ff_bi, ff_wo, ff_bo),
        "rt": (rt_wi, rt_bi, rt_wo, rt_bo),
    }
    in_maps = make_in_maps(np.asarray(local_feat), np.asarray(global_feat),
                           np.asarray(text_feat),
                           np.asarray(w_tl), np.asarray(b_tl),
                           np.asarray(w_tg), np.asarray(b_tg),
                           np.asarray(w_rep), np.asarray(b_rep),
                           np.asarray(pos_local), np.asarray(pos_global),
                           {k: tuple(np.asarray(x) for x in v)
                            for k, v in mha_params.items()})
    res = run_bass_kernel_spmd(nc, in_maps, core_ids=list(range(NCORES)))
    _CACHE["last_results"] = res
    out = np.empty((4, RG, RG), dtype=np.float32)
    for g in range(4):
        out[g] = res.results[2 * g]["logits"]
    return out


# revision 62
# speedup vs baseline: 2.2617x; 1.0372x over previous
"""Trainium2 Bass kernel for nn_Model4 (retrieval_knn).

Model: 3 l2-normalized feature streams -> 4 chained MultiheadAttention blocks
-> full = rt @ t_r.T -> per-group cosine logits [4, 256, 256].

Sharding (8 cores = 4 row-groups x 2 head-halves): core c = 2*g + j.
g owns rows R_g = [256g, 256g+256) (== final group g); j owns qkv feature
columns [512j, 512j+512) for the ff/rt MHAs only.

The collective cost (15us constant + bytes/40GBps, serialized on one device)
dominates, so the design minimizes collectives:
  - tl/tg MHAs: fully replicated within the pair (all 4 heads; K/V computed
    locally over the full sequence).  No collective at all for these two.
  - t_r computed in full on every core; B = wo_rt . t_r and g = t_r.T bo_rt
    precomputed during the AG_E window so the tail is full.T = B.T@ctx + g.
  - ff/rt MHAs: head-half sharded; K/V S-sharded + quad AllGather; context
    halves pair AllGather.  => 4 collectives total.

Mixed precision: fp8e4 (x16 scaling) with DoubleRow matmuls on every path
whose error is damped before the logits (K projections and scores
everywhere; V/context for tl/tg/ff, whose outputs are ~3% of the residual
stream; all wire payloads except ctx_rt).  bf16 is kept on the
logit-critical chain: t_l -> lt -> ff -> V_rt -> ctx_rt -> full, plus t_r,
B, and the final cosine stage.  fp32 PSUM accumulation throughout.

Scale conventions: fp8 weights and activations carry x16; K/V projection
psums therefore carry x256 and are stored as-is; scores carry x4096 (folded
into the exp scale); fp8 AV contexts are rescaled to x16 via a 1/16-valued
broadcast; biases feeding fp8 tensors ship pre-scaled x16 from the host.
The K-projection bias is dropped entirely (softmax is invariant to
per-query constant score shifts); V bias is added to the context after
attention (attention weights sum to 1).
"""
import os
import sys

sys.path.insert(0, "/opt/trn_rl_repo")

import ml_dtypes
import numpy as np

import concourse.bass as bass  # noqa: F401
import concourse.tile as tile
import concourse.mybir as mybir
from concourse import bacc
from concourse.bass_utils import run_bass_kernel_spmd

E = 1024
P = 128
KO = E // P          # 8 feature chunks
RG = 256             # rows per group
NCORES = 8
F32 = mybir.dt.float32
F32R = mybir.dt.float32r
BF16 = mybir.dt.bfloat16
FP8 = mybir.dt.float8e4
U8 = mybir.dt.uint8
DRow = mybir.MatmulPerfMode.DoubleRow
AF = mybir.ActivationFunctionType
GROUPS4 = [[0, 2, 4, 6], [1, 3, 5, 7]]   # gather S-shards across row-groups
GROUPS2 = [[0, 1], [2, 3], [4, 5], [6, 7]]  # exchange head halves within pair
EPS = 1e-8
S16 = 16.0                   # fp8 scale
SC_EXP = 0.0625 / (S16 * S16 * S16 * S16)   # d^-0.5 / (qp x16 . kp x256)
KV8 = 512 * RG               # bytes of one fp8 kp or vp piece
CTXB = 512 * RG * 2          # bytes of one bf16 ctx piece (rt)
VPB = 512 * RG * 2           # bytes of one bf16 vp piece (rt)

_CACHE = {}


def build_nc():
    nc = bacc.Bacc("TRN2", target_bir_lowering=False, debug=False,
                   num_devices=NCORES)
    dram = {}

    def din(name, shape, dt=BF16):
        dram[name] = nc.dram_tensor(name, shape, dt, kind="ExternalInput").ap()

    din("x_text", [E, E])
    din("x_loc", [E, E])
    din("x_glob", [E, E])
    din("x_loc8", [E, E], FP8)   # pre-scaled x16, raw (K path)
    din("x_glob8", [E, E], FP8)  # pre-scaled x16, raw (K path)
    din("x_text_own", [E, RG])
    din("x_loc_own", [E, RG])
    for w in ("w_tl", "w_tg", "w_rep"):
        din(w, [E, E])
    for b in ("b_tl", "b_tg", "b_rep"):
        din(b, [E], F32)
    # Q/K weights fp8 x16 (scores tolerate it); V/out weights bf16.
    # pos embeddings are folded on the host: the K-side term is softmax-
    # invariant (dropped), the V-side term folds into bv.
    for m in ("tl", "tg"):
        din(f"wq_{m}", [E, E], FP8)
        din(f"wk_{m}", [E, E], FP8)
        din(f"wv_{m}", [E, E])
        din(f"wo_{m}", [E, E])
        din(f"bq_{m}", [E], F32)     # x256
        din(f"bo_{m}", [E], F32)     # x1; bv (incl. pos fold) via wo
    for m in ("ff", "rt"):
        din(f"wq_{m}", [E, 512], FP8)
        din(f"wk_{m}", [E, 512], FP8)
        din(f"wv_{m}", [E, 512])
        din(f"wo_{m}", [E, E])
        din(f"bq_{m}", [512], F32)   # x256
        din(f"bo_{m}", [E], F32)     # x1; bv folded via wo
    out_logits = nc.dram_tensor("logits", [RG, RG], F32,
                                kind="ExternalOutput").ap()
    dbg_names = [x for x in os.environ.get("KDEBUG", "").split(",") if x]

    from contextlib import ExitStack
    with tile.TileContext(nc) as tc, ExitStack() as ctx:
        consts = ctx.enter_context(tc.tile_pool(name="consts", bufs=1))
        streams = ctx.enter_context(tc.tile_pool(name="streams", bufs=2))
        kv8s = ctx.enter_context(tc.tile_pool(name="kv8s", bufs=1))
        kvfull = ctx.enter_context(tc.tile_pool(name="kvfull", bufs=1))
        kvbig = ctx.enter_context(tc.tile_pool(name="kvbig", bufs=1))
        kvbf = ctx.enter_context(tc.tile_pool(name="kvbf", bufs=1))
        wfull = ctx.enter_context(tc.tile_pool(name="wfull", bufs=3))
        wfull8 = ctx.enter_context(tc.tile_pool(name="wfull8", bufs=1))
        whalf8 = ctx.enter_context(tc.tile_pool(name="whalf8", bufs=2))
        acts = ctx.enter_context(tc.tile_pool(name="acts", bufs=4))
        acts8 = ctx.enter_context(tc.tile_pool(name="acts8", bufs=2))
        pers = ctx.enter_context(tc.tile_pool(name="pers", bufs=1))
        qps = ctx.enter_context(tc.tile_pool(name="qps", bufs=1))
        exps = ctx.enter_context(tc.tile_pool(name="exps", bufs=2))
        sqs = ctx.enter_context(tc.tile_pool(name="sqs", bufs=2))
        ctxs = ctx.enter_context(tc.tile_pool(name="ctxs", bufs=1))
        bcs = ctx.enter_context(tc.tile_pool(name="bcs", bufs=1))
        smalls = ctx.enter_context(tc.tile_pool(name="smalls", bufs=1))
        ps512 = ctx.enter_context(tc.tile_pool(name="ps512", bufs=3,
                                               space="PSUM"))
        ps256 = ctx.enter_context(tc.tile_pool(name="ps256", bufs=2,
                                               space="PSUM"))
        pssum = ctx.enter_context(tc.tile_pool(name="pssum", bufs=1,
                                               space="PSUM"))
        dram_p = ctx.enter_context(tc.tile_pool(name="dram_p", bufs=1,
                                                space="DRAM"))

        # ---------- constants ----------
        ones_cb = consts.tile([P, 1], BF16)
        nc.vector.memset(ones_cb, 1.0)
        ones_cf = consts.tile([P, 1], F32)
        nc.vector.memset(ones_cf, 1.0)
        ones_col = consts.tile([P, 1], F32R)
        nc.vector.tensor_copy(ones_col, ones_cf)

        def row_const(val):
            tf = consts.tile([1, P], F32, name=f"rc_{val}")
            nc.vector.memset(tf, val)
            tr = consts.tile([1, P], F32R, name=f"rcr_{val}")
            nc.vector.tensor_copy(tr, tf)
            return tr

        ones_row = row_const(1.0)

        def load_bias_pp(name, n):
            t = consts.tile([P, n // P], F32, name=f"c_{name}")
            nc.sync.dma_start(t, dram[name].rearrange("(c p) -> p c", p=P))
            return t

        # ---------- input streams first (gate the first compute) ----------
        def load_stream(name):
            t = streams.tile([P, KO, E], BF16, tag="x", name=name)
            nc.sync.dma_start(t, dram[name].rearrange("(ko p) r -> p ko r",
                                                      p=P))
            return t

        kvg = load_stream("x_glob")            # becomes globn in place
        kvl = load_stream("x_loc")             # becomes localn in place
        kvg8 = kv8s.tile([P, KO, E], FP8, tag="kv8", name="kvg8")
        nc.sync.dma_start(kvg8, dram["x_glob8"].rearrange(
            "(ko p) r -> p ko r", p=P))
        t_own = pers.tile([P, KO, RG], BF16, name="textn_own")
        nc.sync.dma_start(t_own, dram["x_text_own"].rearrange(
            "(ko p) r -> p ko r", p=P))
        l_own = pers.tile([P, KO, RG], BF16, name="localn_own")
        nc.sync.dma_start(l_own, dram["x_loc_own"].rearrange(
            "(ko p) r -> p ko r", p=P))

        bias_pp = {}
        for nm in ("b_tl", "b_tg", "b_rep"):
            bias_pp[nm] = load_bias_pp(nm, E)
        for m in ("tl", "tg"):
            bias_pp[f"bq_{m}"] = load_bias_pp(f"bq_{m}", E)
            bias_pp[f"bo_{m}"] = load_bias_pp(f"bo_{m}", E)
        for m in ("ff", "rt"):
            bias_pp[f"bq_{m}"] = load_bias_pp(f"bq_{m}", 512)
            bias_pp[f"bo_{m}"] = load_bias_pp(f"bo_{m}", E)

        # ---------- debug ----------
        def _dbg(nm, t):
            if nm not in dbg_names:
                return
            do = nc.dram_tensor(f"dbg_{nm}", [P] + list(t.shape[1:]),
                                t.dtype, kind="ExternalOutput").ap()
            nc.sync.dma_start(do, t)

        # ---------- helpers ----------
        def load_w(name, pool, nco, dt=BF16, pad=None):
            t = pool.tile([P, KO, nco * P], dt, tag="w", name=f"w_{name}",
                          padded_shape=[P, KO, pad * P] if pad else None)
            nc.sync.dma_start(t, dram[name].rearrange("(ko p) c -> p ko c",
                                                      p=P))
            return t

        def colsum_inv(src, nko, with_eps=False):
            """src [128, nko, R] bf16: per-free-column 1/||col||, [1,R] f32r."""
            R = src.shape[2]
            inv = smalls.tile([1, R], F32R, tag=f"inv{R}", name="inv")
            for h in range(0, R, 512):
                w = min(512, R - h)
                ps = pssum.tile([1, w], F32, tag=f"cs{w}", name="ps_cs")
                for ko in range(nko):
                    sq = sqs.tile([P, w], BF16, tag=f"sq{w}", name="sq")
                    nc.scalar.activation(sq, src[:, ko, h:h + w], AF.Square)
                    nc.tensor.matmul(ps, ones_cb, sq, start=(ko == 0),
                                     stop=(ko == nko - 1))
                rec = smalls.tile([1, w], F32, tag=f"rc{w}", name="rec")
                with nc.allow_low_precision(reason="fp32 recip+sqrt"):
                    nc.vector.reciprocal(rec, ps)
                if with_eps:
                    # 1/max(ss, eps^2) == min(1/ss, eps^-2), incl. ss == 0
                    nc.vector.tensor_scalar_min(rec, rec,
                                                1.0 / (EPS * EPS))
                nc.scalar.activation(inv[:, h:h + w], rec, AF.Sqrt)
            return inv

        def bcast_row(row_f32r, n, ones=None):
            """[1, n] f32r -> [128, n] f32 broadcast scaled by the ones val."""
            out = bcs.tile([P, n], F32, tag=f"bc{n}", name="bc")
            for h in range(0, n, 512):
                w = min(512, n - h)
                pool, tag = (ps256, "mm") if w <= 256 else (ps512, "mm512")
                ps = pool.tile([P, w], F32, tag=tag, name="ps_bc")
                nc.tensor.matmul(ps, ones or ones_row, row_f32r[:, h:h + w],
                                 start=True, stop=True)
                nc.vector.tensor_copy(out[:, h:h + w], ps)
            return out

        ADD = mybir.AluOpType.add
        MULT = mybir.AluOpType.mult

        def psum_out(o, ps, bias=None, scale=None, residual=None,
                     act_copy=False):
            """PSUM -> SBUF with optional (x scale), (+ bias), (+ residual)."""
            if bias is not None and residual is not None:
                assert scale is None
                nc.vector.scalar_tensor_tensor(o, ps, bias, residual,
                                               ADD, ADD)
            elif bias is not None and scale is not None:
                # (ps + bias) * scale -- biases ship pre-multiplied so that
                # this fused form lands in the target fp8/bf16 scale
                nc.vector.tensor_scalar(o, ps, bias, scale, ADD, MULT)
            elif bias is not None:
                nc.vector.tensor_scalar_add(o, ps, bias)
            elif scale is not None:
                nc.vector.tensor_scalar_mul(o, ps, scale)
            elif act_copy:
                nc.scalar.copy(o, ps)
            else:
                nc.vector.tensor_copy(o, ps)
            if residual is not None and bias is None:
                nc.gpsimd.tensor_add(o, o, residual)

        def gemm_fm(w_sb, act, out, nco, bias=None, scale=None,
                    residual=None, act_copy=False):
            """bf16 feat-major GEMM: out[:,c,:] = w[:,:,c128].T @ act."""
            R = act.shape[2]
            for c in range(nco):
                for h in range(0, R, 512):
                    w = min(512, R - h)
                    pool, tag = (ps256, "mm") if w <= 256 else (ps512, "mm512")
                    ps = pool.tile([P, w], F32, tag=tag, name="ps_g")
                    for ko in range(KO):
                        nc.tensor.matmul(ps, w_sb[:, ko, c * P:(c + 1) * P],
                                         act[:, ko, h:h + w], start=(ko == 0),
                                         stop=(ko == KO - 1))
                    psum_out(out[:, c, h:h + w], ps,
                             bias[:, c:c + 1] if bias is not None else None,
                             scale,
                             residual[:, c, h:h + w] if residual is not None
                             else None, act_copy)

        def gemm_dr(w8, act8, out, nco, bias=None, scale=None,
                    residual=None, act_copy=False, colscale=None):
            """fp8 DoubleRow GEMM (4x PE): same contract as gemm_fm.
            colscale: [128, R] broadcast tile multiplied per output column
            (commuted norm scaling for the K projections)."""
            R = act8.shape[2]
            for c in range(nco):
                for h in range(0, R, 512):
                    w = min(512, R - h)
                    pool, tag = (ps256, "mm") if w <= 256 else (ps512, "mm512")
                    ps = pool.tile([P, w], F32, tag=tag, name="ps_g")
                    for k2 in range(0, KO, 2):
                        nc.tensor.matmul(ps,
                                         w8[:, k2:k2 + 2, c * P:(c + 1) * P],
                                         act8[:, k2:k2 + 2, h:h + w],
                                         start=(k2 == 0), stop=(k2 == KO - 2),
                                         perf_mode=DRow)
                    if colscale is not None:
                        nc.vector.tensor_mul(out[:, c, h:h + w], ps,
                                             colscale[:, h:h + w])
                        continue
                    psum_out(out[:, c, h:h + w], ps,
                             bias[:, c:c + 1] if bias is not None else None,
                             scale,
                             residual[:, c, h:h + w] if residual is not None
                             else None, act_copy)

        def vproj_smajor(w_sb, act, vp, nchan):
            """bf16 S-major V projection: vp[:,s,:] = act[:,:,s128].T @ w."""
            S = act.shape[2]
            for s in range(S // P):
                for h in range(0, nchan, 512):
                    w = min(512, nchan - h)
                    ps = ps512.tile([P, w], F32, tag="mm512", name="ps_v")
                    for ko in range(KO):
                        nc.tensor.matmul(ps, act[:, ko, s * P:(s + 1) * P],
                                         w_sb[:, ko, h:h + w],
                                         start=(ko == 0), stop=(ko == KO - 1))
                    nc.scalar.copy(vp[:, s, h:h + w], ps)

        def attention_mx(qp8, kp2, vp_sl, ctx_out, nheads):
            """fp8 DoubleRow scores + bf16 denominator/AV/context.
            Heads run in interleaved pairs so the cross-engine exp/recip/
            broadcast latencies hide under the other head's matmuls; the
            pair shares one reciprocal and one broadcast.  The V bias is
            folded into the out-projection bias on the host."""
            for hp in range(0, nheads, 2):
                pss = pssum.tile([1, 2 * RG], F32, tag="cs512", name="ps_sm")
                expts = []
                for i in range(2):
                    h = hp + i
                    expt = exps.tile([P, KO, RG], BF16, tag="expb",
                                     name=f"exptb{i}")
                    for s in range(KO):
                        ps = ps256.tile([P, RG], F32, tag="mm", name="ps_sc")
                        nc.tensor.matmul(ps, kp2(h, s),
                                         qp8[:, 2 * h:2 * h + 2],
                                         start=True, stop=True,
                                         perf_mode=DRow)
                        nc.scalar.activation(expt[:, s], ps, AF.Exp,
                                             scale=SC_EXP)
                    expts.append(expt)
                for i in range(2):
                    for s in range(KO):
                        nc.tensor.matmul(pss[:, i * RG:(i + 1) * RG],
                                         ones_cb, expts[i][:, s],
                                         start=(s == 0), stop=(s == KO - 1))
                inv = smalls.tile([1, 2 * RG], F32R, tag="invsm",
                                  name="inv_sm")
                with nc.allow_low_precision(reason="fp32r rounding intended"):
                    nc.vector.reciprocal(inv, pss)
                bc = bcast_row(inv, 2 * RG)
                for i in range(2):
                    h = hp + i
                    for dk in range(2):
                        cc = 2 * h + dk
                        ps = ps256.tile([P, RG], F32, tag="mm", name="ps_av")
                        for s in range(KO):
                            nc.tensor.matmul(ps, vp_sl(s, cc),
                                             expts[i][:, s],
                                             start=(s == 0),
                                             stop=(s == KO - 1))
                        nc.vector.tensor_mul(ctx_out[:, cc], ps,
                                             bc[:, i * RG:(i + 1) * RG])

        def pack_piece(inbuf, off, sb_tile):
            """SBUF tile -> byte-typed dram flat buffer (uint8 bitcast)."""
            t = sb_tile.bitcast(U8)
            shp = t.shape
            n = P * shp[1] * shp[2]
            nc.sync.dma_start(
                inbuf[off:off + n].rearrange("(p a b) -> p a b", p=P,
                                             a=shp[1]), t)

        def allgather(inbuf, outbuf, groups):
            nc.gpsimd.collective_compute(
                "AllGather", mybir.AluOpType.bypass,
                replica_groups=groups,
                ins=[inbuf.opt()], outs=[outbuf.opt()])

        # ---------- stage 0: normalize ----------
        # glob first (feeds the tg chain = critical path).  The bf16 master
        # is normalized in place (V path); the raw fp8 copy feeds the K
        # projection, whose column norm scaling commutes to the kp output.
        inv_g = colsum_inv(kvg, KO)
        bc_g = bcast_row(inv_g, E)
        for ko in range(KO):
            nc.vector.tensor_mul(kvg[:, ko], kvg[:, ko], bc_g)
        inv_l = colsum_inv(kvl, KO)
        bc_l = bcast_row(inv_l, E)
        for ko in range(KO):
            nc.vector.tensor_mul(kvl[:, ko], kvl[:, ko], bc_l)
        inv_to = colsum_inv(t_own, KO)
        bc_to = bcast_row(inv_to, RG)
        for ko in range(KO):
            nc.vector.tensor_mul(t_own[:, ko], t_own[:, ko], bc_to)
        inv_lo = colsum_inv(l_own, KO)
        bc_lo = bcast_row(inv_lo, RG)
        for ko in range(KO):
            nc.vector.tensor_mul(l_own[:, ko], l_own[:, ko], bc_lo)
        # full-text norms via chunked DMA (all Sqrt before the first Exp)
        inv_t = smalls.tile([1, E], F32R, tag="inv1024", name="inv_t")
        for h in range(2):
            ps = pssum.tile([1, 512], F32, tag="cs512", name="ps_xt")
            for ko in range(KO):
                stg = sqs.tile([P, 512], BF16, tag="stg", name="stg")
                nc.sync.dma_start(
                    stg, dram["x_text"][ko * P:(ko + 1) * P,
                                        h * 512:(h + 1) * 512])
                sq = sqs.tile([P, 512], BF16, tag="sq512", name="sq")
                nc.scalar.activation(sq, stg, AF.Square)
                nc.tensor.matmul(ps, ones_cb, sq, start=(ko == 0),
                                 stop=(ko == KO - 1))
            rec = smalls.tile([1, 512], F32, tag="rc512", name="rec_t")
            with nc.allow_low_precision(reason="fp32 recip+sqrt"):
                nc.vector.reciprocal(rec, ps)
            nc.scalar.activation(inv_t[:, h * 512:(h + 1) * 512], rec,
                                 AF.Sqrt)

        _dbg("kvg8", kvg8)
        _dbg("t_own", t_own)
        _dbg("l_own", l_own)

        # ---------- tg chain (critical path to the first collective) ------
        wk_tg = load_w("wk_tg", wfull8, 8, FP8)
        kp_tg = kvfull.tile([P, KO, E], FP8, tag="kp8", name="kp_tg")
        gemm_dr(wk_tg, kvg8, kp_tg, KO, colscale=bc_g)
        wv_tg = load_w("wv_tg", wfull, 8)
        vp_tg = kvbig.tile([P, KO, E], BF16, tag="vpb", name="vp_tg")
        vproj_smajor(wv_tg, kvg, vp_tg, E)

        w_tg = load_w("w_tg", wfull, 8)
        t_g = acts.tile([P, KO, RG], BF16, tag="act", name="t_g")
        gemm_fm(w_tg, t_own, t_g, KO, bias=bias_pp["b_tg"])
        t_g8 = acts8.tile([P, KO, RG], FP8, tag="act8", name="t_g8")
        for ko in range(KO):
            nc.scalar.activation(t_g8[:, ko], t_g[:, ko], AF.Copy, scale=S16)
        wq_tg = load_w("wq_tg", wfull8, 8, FP8)
        qp_tg = qps.tile([P, KO, RG], FP8, tag="qp", name="qp_tg")
        gemm_dr(wq_tg, t_g8, qp_tg, KO, bias=bias_pp["bq_tg"],
                scale=1.0 / S16)

        ctx_tg = ctxs.tile([P, KO, RG], BF16, tag="cf", name="ctx_tg")
        attention_mx(qp_tg,
                     lambda h, s: kp_tg[:, 2 * h:2 * h + 2,
                                        s * P:(s + 1) * P],
                     lambda s, cc: vp_tg[:, s, cc * P:(cc + 1) * P],
                     ctx_tg, 4)
        wo_tg = load_w("wo_tg", wfull, 8)
        gt = acts.tile([P, KO, RG], BF16, tag="act", name="gt")
        gemm_fm(wo_tg, ctx_tg, gt, KO, bias=bias_pp["bo_tg"], residual=t_g)
        gt8 = acts8.tile([P, KO, RG], FP8, tag="act8", name="gt8")
        for ko in range(KO):
            nc.scalar.activation(gt8[:, ko], gt[:, ko], AF.Copy, scale=S16)

        # ff K/V S-shard + quad AllGather (AG_C) -- issue ASAP
        wk_ff = load_w("wk_ff", whalf8, 4, FP8)
        kp_ff = ctxs.tile([P, 4, RG], FP8, tag="kp", name="kp_ff")
        gemm_dr(wk_ff, gt8, kp_ff, 4, act_copy=True)
        wv_ff = load_w("wv_ff", wfull, 4, pad=8)
        vp_ff = ctxs.tile([P, 2, 512], BF16, tag="vp", name="vp_ff")
        vproj_smajor(wv_ff, gt, vp_ff, 512)
        _dbg("kp_ff_piece", kp_ff)
        _dbg("vp_ff_piece", vp_ff)
        in_ff = dram_p.tile([KV8 + VPB], U8, name="in_ff")
        out_ff = dram_p.tile([4, KV8 + VPB], U8, name="out_ff")
        pack_piece(in_ff, 0, kp_ff)
        pack_piece(in_ff, KV8, vp_ff)
        allgather(in_ff, out_ff, GROUPS4)

        # ---------- tl chain (runs on the PE under AG_C) ----------
        # kvl8 shares kvg8's slab; its DMA is issued only here so the SP
        # queue can't head-of-line block the tg-chain weight loads on it
        kvl8 = kv8s.tile([P, KO, E], FP8, tag="kv8", name="kvl8")
        nc.sync.dma_start(kvl8, dram["x_loc8"].rearrange(
            "(ko p) r -> p ko r", p=P))
        wk_tl = load_w("wk_tl", wfull8, 8, FP8)
        kp_tl = kvfull.tile([P, KO, E], FP8, tag="kp8", name="kp_tl")
        gemm_dr(wk_tl, kvl8, kp_tl, KO, colscale=bc_l)
        wv_tl = load_w("wv_tl", wfull, 8)
        vp_tl = kvbig.tile([P, KO, E], BF16, tag="vpb", name="vp_tl")
        vproj_smajor(wv_tl, kvl, vp_tl, E)

        w_tl = load_w("w_tl", wfull, 8)
        t_l = acts.tile([P, KO, RG], BF16, tag="act", name="t_l")
        gemm_fm(w_tl, t_own, t_l, KO, bias=bias_pp["b_tl"])
        t_l8 = acts8.tile([P, KO, RG], FP8, tag="act8", name="t_l8")
        for ko in range(KO):
            nc.scalar.activation(t_l8[:, ko], t_l[:, ko], AF.Copy, scale=S16)
        wq_tl = load_w("wq_tl", wfull8, 8, FP8)
        qp_tl = qps.tile([P, KO, RG], FP8, tag="qp", name="qp_tl")
        gemm_dr(wq_tl, t_l8, qp_tl, KO, bias=bias_pp["bq_tl"],
                scale=1.0 / S16)

        ctx_tl = ctxs.tile([P, KO, RG], BF16, tag="cf", name="ctx_tl")
        attention_mx(qp_tl,
                     lambda h, s: kp_tl[:, 2 * h:2 * h + 2,
                                        s * P:(s + 1) * P],
                     lambda s, cc: vp_tl[:, s, cc * P:(cc + 1) * P],
                     ctx_tl, 4)
        wo_tl = load_w("wo_tl", wfull, 8)
        lt = acts.tile([P, KO, RG], BF16, tag="act", name="lt")
        gemm_fm(wo_tl, ctx_tl, lt, KO, bias=bias_pp["bo_tl"], residual=t_l)
        lt8 = acts8.tile([P, KO, RG], FP8, tag="act8", name="lt8")
        for ko in range(KO):
            nc.scalar.activation(lt8[:, ko], lt[:, ko], AF.Copy, scale=S16)
        wq_ff = load_w("wq_ff", whalf8, 4, FP8)
        qp_ff = qps.tile([P, 4, RG], FP8, tag="qph", name="qp_ff")
        gemm_dr(wq_ff, lt8, qp_ff, 4, bias=bias_pp["bq_ff"], scale=1.0 / S16)

        _dbg("t_l", t_l)
        _dbg("lt", lt)

        # ---------- ff attention (waits on AG_C) ----------
        kpf_ff = kvfull.tile([P, 4, 4, RG], FP8, tag="kp8", name="kpf_ff",
                             padded_shape=[P, 4, 4, 2 * RG])
        vpf_ff = kvbig.tile([P, 4, 2, 512], BF16, tag="vpb", name="vpf_ff",
                            padded_shape=[P, 4, 2, 1024])
        for gs in range(4):
            nc.sync.dma_start(
                kpf_ff[:, gs].bitcast(U8),
                out_ff[gs, 0:KV8].rearrange("(p a b) -> p a b", p=P, a=4))
            nc.sync.dma_start(
                vpf_ff[:, gs].bitcast(U8),
                out_ff[gs, KV8:].rearrange("(p a b) -> p a b", p=P, a=2))
        _dbg("kpf_ff", kpf_ff)
        ctxh_ff = ctxs.tile([P, 4, RG], BF16, tag="cf", name="ctxh_ff",
                            padded_shape=[P, 4, 2 * RG])
        attention_mx(qp_ff,
                     lambda h, s: kpf_ff[:, s // 2, 2 * h:2 * h + 2,
                                         (s % 2) * P:(s % 2 + 1) * P],
                     lambda s, cc: vpf_ff[:, s // 2, s % 2,
                                          cc * P:(cc + 1) * P],
                     ctxh_ff, 2)
        _dbg("ctxh_ff", ctxh_ff)
        in_cff = dram_p.tile([CTXB], U8, name="in_cff")
        out_cff = dram_p.tile([2, CTXB], U8, name="out_cff")
        pack_piece(in_cff, 0, ctxh_ff)
        allgather(in_cff, out_cff, GROUPS2)

        # t_r (full rows) + qp_rt fill the AG_D window
        xt = load_stream("x_text")
        bc_t = bcast_row(inv_t, E)
        for ko in range(KO):
            nc.vector.tensor_mul(xt[:, ko], xt[:, ko], bc_t)
        w_rep = load_w("w_rep", wfull, 8)
        t_r = kvbig.tile([P, KO, E], BF16, tag="vpb", name="t_r")
        gemm_fm(w_rep, xt, t_r, KO, bias=bias_pp["b_rep"])
        wq_rt = load_w("wq_rt", whalf8, 4, FP8)
        t_r_own8 = acts8.tile([P, KO, RG], FP8, tag="act8", name="t_r_own8")
        gemm_fm(w_rep, t_own, t_r_own8, KO, bias=bias_pp["b_rep"], scale=S16)
        qp_rt = qps.tile([P, 4, RG], FP8, tag="qph", name="qp_rt")
        gemm_dr(wq_rt, t_r_own8, qp_rt, 4, bias=bias_pp["bq_rt"],
                scale=1.0 / S16)

        # ff out-projection + residual -> ff activation (bf16 + fp8 copy)
        ctxf_ff = ctxs.tile([P, KO, RG], BF16, tag="cf", name="ctxf_ff")
        for r in range(2):
            nc.sync.dma_start(
                ctxf_ff[:, 4 * r:4 * r + 4].bitcast(U8),
                out_cff[r].rearrange("(p a b) -> p a b", p=P, a=4))
        wo_ff = load_w("wo_ff", wfull, 8)
        ffa = acts.tile([P, KO, RG], BF16, tag="act", name="ffa")
        gemm_fm(wo_ff, ctxf_ff, ffa, KO, bias=bias_pp["bo_ff"], residual=lt)
        ffa8 = acts8.tile([P, KO, RG], FP8, tag="act8", name="ffa8")
        for ko in range(KO):
            nc.scalar.activation(ffa8[:, ko], ffa[:, ko], AF.Copy, scale=S16)
        _dbg("ffa", ffa)

        # rt K/V S-shard + quad AllGather (AG_E): K fp8, V bf16
        wk_rt = load_w("wk_rt", whalf8, 4, FP8)
        kp_rt = ctxs.tile([P, 4, RG], FP8, tag="kp", name="kp_rt")
        gemm_dr(wk_rt, ffa8, kp_rt, 4, act_copy=True)
        wv_rt = load_w("wv_rt", wfull, 4, pad=8)
        vp_rt = kvbf.tile([P, 2, 512], BF16, tag="rtkv", name="vp_rt",
                          padded_shape=[P, 2, 2048])
        vproj_smajor(wv_rt, ffa, vp_rt, 512)
        in_rt = dram_p.tile([KV8 + VPB], U8, name="in_rt")
        out_rt = dram_p.tile([4, KV8 + VPB], U8, name="out_rt")
        pack_piece(in_rt, 0, kp_rt)
        pack_piece(in_rt, KV8, vp_rt)
        allgather(in_rt, out_rt, GROUPS4)

        # B = wo_rt . t_r and g = t_r.T bo_rt fill the AG_E window, so the
        # post-AG_F tail needs no rt out-projection: full.T = B.T@ctx_rt + g
        wo_rt_fm = load_w("wo_rt", wfull, 8)
        Bm = streams.tile([P, KO, E], BF16, tag="x", name="Bmat")
        gemm_fm(wo_rt_fm, t_r, Bm, KO)
        bo_rt_bf = consts.tile([P, KO], BF16, name="bo_rt_bf")
        nc.vector.tensor_copy(bo_rt_bf, bias_pp["bo_rt"])
        g_pp = consts.tile([P, KO], F32, name="g_pp")
        for nch in range(KO):
            psg = pssum.tile([P, 1], F32, tag="g1", name="ps_g1")
            for ko in range(KO):
                nc.tensor.matmul(psg, t_r[:, ko, nch * P:(nch + 1) * P],
                                 bo_rt_bf[:, ko:ko + 1], start=(ko == 0),
                                 stop=(ko == KO - 1))
            nc.vector.tensor_copy(g_pp[:, nch:nch + 1], psg)

        # ---------- rt attention (waits on AG_E) ----------
        kpf_rt = kvfull.tile([P, 4, 4, RG], FP8, tag="kp8", name="kpf_rt",
                             padded_shape=[P, 4, 4, 2 * RG])
        vpf_rt = kvbf.tile([P, 4, 2, 512], BF16, tag="rtkv",
                           name="vpf_rt")
        for gs in range(4):
            nc.sync.dma_start(
                kpf_rt[:, gs].bitcast(U8),
                out_rt[gs, 0:KV8].rearrange("(p a b) -> p a b", p=P, a=4))
            nc.sync.dma_start(
                vpf_rt[:, gs].bitcast(U8),
                out_rt[gs, KV8:].rearrange("(p a b) -> p a b", p=P, a=2))
        ctxh_rt = acts.tile([P, 4, RG], BF16, tag="act", name="ctxh_rt",
                            padded_shape=[P, 4, 2 * RG])
        attention_mx(qp_rt,
                     lambda h, s: kpf_rt[:, s // 2, 2 * h:2 * h + 2,
                                         (s % 2) * P:(s % 2 + 1) * P],
                     lambda s, cc: vpf_rt[:, s // 2, s % 2,
                                          cc * P:(cc + 1) * P],
                     ctxh_rt, 2)
        _dbg("ctxh_rt", ctxh_rt)
        in_crt = dram_p.tile([CTXB], U8, name="in_crt")
        out_crt = dram_p.tile([2, CTXB], U8, name="out_crt")
        pack_piece(in_crt, 0, ctxh_rt)
        allgather(in_crt, out_crt, GROUPS2)

        ctxf_rt = kvbf.tile([P, KO, RG], BF16, tag="rtkv", name="ctxf_rt",
                            padded_shape=[P, KO, 2 * RG])
        for r in range(2):
            nc.sync.dma_start(
                ctxf_rt[:, 4 * r:4 * r + 4].bitcast(U8),
                out_crt[r].rearrange("(p a b) -> p a b", p=P, a=4))

        # ---------- full.T = B.T @ ctx_rt + g, cosine logits ----------
        fullT = acts.tile([P, KO, RG], BF16, tag="act", name="fullT")
        for nchunk in range(KO):
            ps = ps256.tile([P, RG], F32, tag="mm", name="ps_full")
            for cc in range(KO):
                nc.tensor.matmul(ps, Bm[:, cc, nchunk * P:(nchunk + 1) * P],
                                 ctxf_rt[:, cc], start=(cc == 0),
                                 stop=(cc == KO - 1))
            nc.vector.tensor_scalar_add(fullT[:, nchunk], ps,
                                        g_pp[:, nchunk:nchunk + 1])

        inv_full = colsum_inv(fullT, KO, with_eps=True)
        # transpose the per-row inverse norms to a per-partition layout via
        # a tiny DRAM roundtrip, then fold the cosine normalization into the
        # logits copy-out as a per-partition scale
        invq_d = dram_p.tile([RG], F32, name="invq_d")
        nc.sync.dma_start(invq_d, inv_full.bitcast(F32))
        invq = smalls.tile([P, 2], F32, tag="invq", name="invq")
        nc.sync.dma_start(invq, invq_d.rearrange("(c p) -> p c", p=P))

        lg = bcs.tile([P, 2, RG], F32, tag="bc1024", name="lg",
                      padded_shape=[P, 2, 2 * RG])
        for lc in range(2):
            ps = ps256.tile([P, RG], F32, tag="mm", name="ps_lg")
            for ko in range(KO):
                nc.tensor.matmul(ps, fullT[:, ko, lc * P:(lc + 1) * P],
                                 l_own[:, ko], start=(ko == 0),
                                 stop=(ko == KO - 1))
            nc.vector.tensor_scalar_mul(lg[:, lc], ps, invq[:, lc:lc + 1])
        nc.sync.dma_start(out_logits.rearrange("(lc p) q -> p lc q", p=P), lg)

    nc.compile()
    return nc


def make_in_maps(local_feat, global_feat, text_feat,
                 w_tl, b_tl, w_tg, b_tg, w_rep, b_rep,
                 pos_local, pos_global, mha_params):
    """mha_params: dict m -> (wi, bi, wo, bo)."""
    f32 = np.float32
    bf16 = ml_dtypes.bfloat16
    fp8 = ml_dtypes.float8_e4m3

    def tb(x):
        return np.ascontiguousarray(np.asarray(x).T.astype(bf16))

    def t8(x):
        return np.ascontiguousarray(
            (np.asarray(x).T.astype(f32) * 16.0).astype(fp8))

    textT = tb(text_feat)
    locT = tb(local_feat)
    shared = {
        "x_text": textT, "x_loc": locT, "x_glob": tb(global_feat),
        "x_loc8": t8(local_feat), "x_glob8": t8(global_feat),
        "w_tl": tb(w_tl), "w_tg": tb(w_tg), "w_rep": tb(w_rep),
        "b_tl": b_tl.astype(f32), "b_tg": b_tg.astype(f32),
        "b_rep": b_rep.astype(f32),
    }
    pos = {"tl": np.asarray(pos_local, dtype=np.float64),
           "tg": np.asarray(pos_global, dtype=np.float64)}
    for m in ("tl", "tg"):
        wi, bi, wo, bo = mha_params[m]
        shared[f"wq_{m}"] = t8(wi[0 * E:1 * E])
        shared[f"wk_{m}"] = t8(wi[1 * E:2 * E])
        shared[f"wv_{m}"] = tb(wi[2 * E:3 * E])
        shared[f"wo_{m}"] = tb(wo)
        shared[f"bq_{m}"] = bi[0 * E:1 * E].astype(f32) * 256.0
        # parameter-only folds: the V-side pos term joins bv, and the
        # whole V bias is pushed through the out-projection into bo; the
        # K-side pos term is softmax-invariant and dropped
        bv_fold = (np.asarray(bi[2 * E:3 * E], dtype=np.float64)
                   + pos[m] @ np.asarray(wi[2 * E:3 * E],
                                         dtype=np.float64).T)
        shared[f"bo_{m}"] = (np.asarray(bo, dtype=np.float64)
                             + bv_fold
                             @ np.asarray(wo, dtype=np.float64).T
                             ).astype(f32)
    per_j = {}
    for j in range(2):
        d = {}
        sl = slice(512 * j, 512 * (j + 1))
        for m in ("ff", "rt"):
            wi, bi, wo, bo = mha_params[m]
            d[f"wq_{m}"] = t8(wi[0 * E:1 * E][sl])
            d[f"wk_{m}"] = t8(wi[1 * E:2 * E][sl])
            d[f"wv_{m}"] = tb(wi[2 * E:3 * E][sl])
            if m == "rt":
                # wo_rt is consumed as B = wo_rt . t_r (contraction over
                # the output-feature axis), so it ships untransposed
                d[f"wo_{m}"] = np.ascontiguousarray(
                    np.asarray(wo).astype(bf16))
            else:
                d[f"wo_{m}"] = tb(wo)
            d[f"bq_{m}"] = bi[0 * E:1 * E][sl].astype(f32) * 256.0
            d[f"bo_{m}"] = (np.asarray(bo, dtype=np.float64)
                            + np.asarray(bi[2 * E:3 * E], dtype=np.float64)
                            @ np.asarray(wo, dtype=np.float64).T
                            ).astype(f32)
        per_j[j] = d

    in_maps = []
    for c in range(NCORES):
        g, j = c // 2, c % 2
        rs = slice(RG * g, RG * (g + 1))
        m = {
            "x_text_own": np.ascontiguousarray(textT[:, rs]),
            "x_loc_own": np.ascontiguousarray(locT[:, rs]),
        }
        m.update(shared)
        m.update(per_j[j])
        in_maps.append(m)
    return in_maps


def kernel(local_feat, global_feat, text_feat,
           w_tl, b_tl, w_tg, b_tg, w_rep, b_rep,
           pos_local, pos_global,
           tl_wi, tl_bi, tl_wo, tl_bo,
           tg_wi, tg_bi, tg_wo, tg_bo,
           ff_wi, ff_bi, ff_wo, ff_bo,
           rt_wi, rt_bi, rt_wo, rt_bo,
           n_groups):
    assert int(n_groups) == 4
    if "nc" not in _CACHE:
        _CACHE["nc"] = build_nc()
    nc = _CACHE["nc"]
    mha_params = {
        "tl": (tl_wi, tl_bi, tl_wo, tl_bo),
        "tg": (tg_wi, tg_bi, tg_wo, tg_bo),
        "ff": (ff_wi, ff_bi, ff_wo, ff_bo),
        "rt": (rt_wi, rt_bi, rt_wo, rt_bo),
    }
    in_maps = make_in_maps(np.asarray(local_feat), np.asarray(global_feat),
                           np.asarray(text_feat),
                           np.asarray(w_tl), np.asarray(b_tl),
                           np.asarray(w_tg), np.asarray(b_tg),
                           np.asarray(w_rep), np.asarray(b_rep),
                           np.asarray(pos_local), np.asarray(pos_global),
                           {k: tuple(np.asarray(x) for x in v)
                            for k, v in mha_params.items()})
    res = run_bass_kernel_spmd(nc, in_maps, core_ids=list(range(NCORES)))
    _CACHE["last_results"] = res
    out = np.empty((4, RG, RG), dtype=np.float32)
    for g in range(4):
        out[g] = res.results[2 * g]["logits"]
    return out
